# revision 28
# baseline (speedup 1.0000x reference)
"""Trainium2 Bass kernel for a dense transformer encoder layer.

Reference computation (per batch b):
    q = x.reshape(L, H, E)                       # H=16 heads, E=64
    scores = q @ q^T per head, scaled softmax    # A = softmax(s/8)
    new_x  = concat_h(A_h @ q_h)                 # [L, D]
    x1 = LN(x + new_x; g1, be1)
    y  = relu(x1 @ w1^T + b1) @ w2^T + b2
    out = LN(x1 + y; g2, be2)

Sharding: pure data parallel over (batch, seq-half): core c handles
batch c//2, query rows [(c%2)*1024, +1024).  Keys/values span the full
sequence of that batch, so every core gets the whole x[b] (queries
reordered first) and the full FFN weights.  No device collectives.

v2 design notes (all matmuls bf16, PE does ONLY matmuls):
  - x^T, U^T, x1^T are produced with DMA xbar transposes (16-bit dtype,
    src partition %16, free %128), not PE transposes.
  - scores are computed TRANSPOSED ([s, l]) so exp(scores^T) is the
    moving operand of the AV matmul; V carries a ones column so the
    softmax denominator rides along in row 64 of U^T (rows 65..79 pad
    to the xbar 16-row granularity with zero columns).
  - FFN weights are streamed from HBM once, as one [128, 1024] stripe
    per 128-row block (64 DMAs total instead of 1024 tile DMAs).
  - FFN1 accumulates over d-chunks with the stationary w1 tile reused
    across both 512-wide moving slabs; FFN2 uses h^T tiles as the
    stationary and w2 stripes as the moving operand, producing y
    ROW-major directly into PSUM (no output transpose at all).
"""

import numpy as np

import concourse.bass as bass
import concourse.tile as tile
from concourse import bacc
from concourse import mybir

F32 = mybir.dt.float32
BF16 = mybir.dt.bfloat16
EXP = mybir.ActivationFunctionType.Exp
RELU = mybir.ActivationFunctionType.Relu
SQRT = mybir.ActivationFunctionType.Sqrt
SQUARE = mybir.ActivationFunctionType.Square
IDENT = mybir.ActivationFunctionType.Identity
ADD = mybir.AluOpType.add
SUB = mybir.AluOpType.subtract
MUL = mybir.AluOpType.mult

LN_EPS = 1e-5
E = 64          # head dim
W = 80          # head dim + ones column + pad to xbar 16-row granularity
P = 128         # partitions


def build_program(S=2048, D=1024, F=4096):
    """Per-core program.  S = full seq len; queries are rows [0, Lq)."""
    H = D // E
    Lq = S // 2
    ST = S // P          # key tiles
    LT = Lq // P         # query row tiles
    DT = D // P          # d chunks
    FT = F // P          # f chunks
    NSL = 2
    SL = Lq // NSL       # moving slab width (512)
    GS = min(512, D)     # bn_stats subgroup size

    nc = bacc.Bacc("TRN2")

    xb = nc.dram_tensor("xb", [S, D], F32, kind="ExternalInput")
    xb16 = nc.dram_tensor("xb16", [S, D], BF16, kind="ExternalInput")
    w1s = nc.dram_tensor("w1s", [FT, P, D], BF16, kind="ExternalInput")
    w2s = nc.dram_tensor("w2s", [FT, P, D], BF16, kind="ExternalInput")
    b1 = nc.dram_tensor("b1", [F], F32, kind="ExternalInput")
    b2 = nc.dram_tensor("b2", [D], F32, kind="ExternalInput")
    g1h = nc.dram_tensor("g1h", [D], BF16, kind="ExternalInput")
    be1h = nc.dram_tensor("be1h", [D], BF16, kind="ExternalInput")
    g2 = nc.dram_tensor("g2", [D], F32, kind="ExternalInput")
    be2 = nc.dram_tensor("be2", [D], F32, kind="ExternalInput")
    out = nc.dram_tensor("out", [Lq, D], F32, kind="ExternalOutput")

    def bcast(dram_vec, n):
        a = dram_vec[:]
        return bass.AP(tensor=a.tensor, offset=a.offset, ap=[[0, P]] + a.ap)

    with tile.TileContext(nc) as tc:
        with (
            tc.tile_pool(name="persist", bufs=1) as persist,
            tc.tile_pool(name="small", bufs=6) as small,
            tc.tile_pool(name="gb", bufs=1) as gbp,
            tc.tile_pool(name="resp", bufs=2) as resp,
        ):
            b1s = persist.tile([P, FT], F32)
            nc.sync.dma_start(out=b1s, in_=b1[:].rearrange("(t p) -> p t", p=P))
            epst = persist.tile([P, 1], F32)
            nc.vector.memset(epst, LN_EPS)
            # x1 (post-LN1) in bf16: residual-2 source and FFN1 input
            x1b = persist.tile([P, LT, D], BF16)
            # x1^T: [p, lt, dc, j] = x1[lt*128+j, dc*128+p]
            x1T = persist.tile([P, LT, DT, P], BF16)

            # ---------------- attention ----------------
            with (
                tc.tile_pool(name="attn_sb", bufs=1) as asb,
                tc.tile_pool(name="xrp", bufs=3) as xrp,
                tc.tile_pool(name="etp", bufs=3) as etp,
                tc.tile_pool(name="utsp", bufs=2) as utsp,
                tc.tile_pool(name="usp", bufs=3) as usp,
                tc.tile_pool(name="recp", bufs=4) as recp,
                tc.tile_pool(name="lnsc", bufs=3) as lnsc,
            ):
                # attention output, bf16 (residual add upcasts later)
                new_x = asb.tile([P, LT, D], BF16)
                # x^T tiles: [P, DT, S]; d-chunk t holds heads 2t, 2t+1.
                # Interleave the transposes with the vaug row loads so the
                # first heads' operands land as early as possible.
                xT = asb.tile([P, DT, S], BF16)
                vaug = asb.tile([P, ST, H, W], BF16)
                # query rows (bf16) kept for the LN1 residual
                xrows = asb.tile([P, LT, D], BF16)
                nc.gpsimd.memset(vaug[:, :, :, E:W], 0.0)
                nc.gpsimd.memset(vaug[:, :, :, E:E + 1], 1.0)
                nc.sync.dma_start_transpose(
                    out=xT[:, 0, :], in_=xb16[:, 0:P])
                for u in range(ST):
                    if u < LT:
                        xr = xrows[:, u, :]
                    else:
                        xr = xrp.tile([P, D], BF16, tag="xr")
                    nc.sync.dma_start(out=xr, in_=xb16[u * P:(u + 1) * P, :])
                    nc.vector.tensor_copy(
                        out=vaug[:, u, :, 0:E],
                        in_=xr.rearrange("p (h e) -> p h e", e=E))
                    if u % 2 == 1 and 1 + u // 2 < DT:
                        t = 1 + u // 2
                        nc.sync.dma_start_transpose(
                            out=xT[:, t, :], in_=xb16[:, t * P:(t + 1) * P])

                g1b = gbp.tile([P, D], BF16, tag="g1")
                nc.gpsimd.dma_start(out=g1b, in_=bcast(g1h, D))
                be1b = gbp.tile([P, D], BF16, tag="be1")
                nc.gpsimd.dma_start(out=be1b, in_=bcast(be1h, D))

                def head_epilogue(h, ut):
                    uts = utsp.tile([W, Lq], BF16, name="uts", tag="uts")
                    nc.vector.tensor_copy(out=uts, in_=ut)
                    # U: [p, lt, w] = U^T[w, lt*128+p]
                    us = usp.tile([P, LT, W], BF16, name="us", tag="us")
                    nc.sync.dma_start_transpose(out=us, in_=uts)
                    rec = recp.tile([P, LT], F32, name="rec", tag="rec")
                    nc.vector.reciprocal(out=rec, in_=us[:, :, E])
                    for lt in range(LT):
                        nc.vector.tensor_scalar_mul(
                            out=new_x[:, lt, h * E:(h + 1) * E],
                            in0=us[:, lt, 0:E],
                            scalar1=rec[:, lt:lt + 1])

                with (
                    tc.tile_pool(name="scp", bufs=2, space="PSUM") as scp,
                    tc.tile_pool(name="utp", bufs=2, space="PSUM") as utp,
                ):
                    # AV lags TWO chunks behind scores/exp and flows across
                    # head boundaries, so neither the PE nor ACT ever stalls
                    # on the other inside the attention loop.
                    pend = []

                    def emit_one():
                        et_p, u_p, ut_p, h_p = pend.pop(0)
                        for s in range(NSL):
                            nc.tensor.matmul(
                                ut_p[:, s * SL:(s + 1) * SL],
                                vaug[:, u_p, h_p, :],
                                et_p[:, s * SL:(s + 1) * SL],
                                start=(u_p == 0), stop=(u_p == ST - 1))
                        if u_p == ST - 1:
                            head_epilogue(h_p, ut_p)

                    for h in range(H):
                        t, ro = h // 2, (h % 2) * E
                        ut = utp.tile([W, Lq], F32, name="ut", tag="ut")
                        for u in range(ST):
                            sc = scp.tile([P, Lq], F32, name="sc", tag="sc")
                            et = etp.tile([P, Lq], BF16, name="et", tag="et")
                            for s in range(NSL):
                                nc.tensor.matmul(
                                    sc[:, s * SL:(s + 1) * SL],
                                    xT[ro:ro + E, t, u * P:(u + 1) * P],
                                    xT[ro:ro + E, t, s * SL:(s + 1) * SL],
                                    start=True, stop=True)
                            if len(pend) >= 2:
                                emit_one()
                            nc.scalar.activation(
                                out=et, in_=sc, func=EXP, scale=1.0 / 8.0)
                            pend.append((et, u, ut, h))
                    while pend:
                        emit_one()

                # residual 1 + LN1 -> x1b (bf16) and x1T (dma transpose).
                # Work is spread over DVE / ACT / GpSimd and pipelined
                # across row tiles to shorten the attention->FFN boundary.
                for lt in range(LT):
                    xs = resp.tile([P, D], F32, tag="res")
                    rs = small.tile([P, 1], F32, tag="rs")
                    nc.vector.scalar_tensor_tensor(
                        out=xs, in0=new_x[:, lt, :], scalar=1.0,
                        in1=xrows[:, lt, :], op0=MUL, op1=ADD, accum_out=rs)
                    mean = small.tile([P, 1], F32, tag="mean")
                    nc.vector.tensor_scalar_mul(
                        out=mean, in0=rs, scalar1=1.0 / D)
                    sq = lnsc.tile([P, D], F32, tag="sq", bufs=1)
                    ssq = small.tile([P, 1], F32, tag="ssq")
                    nc.scalar.activation(
                        out=sq, in_=xs, func=SQUARE, accum_out=ssq)
                    nmsq = small.tile([P, 1], F32, tag="nmsq")
                    # var = ssq/D - mean^2  (+eps under the sqrt)
                    nc.vector.scalar_tensor_tensor(
                        out=nmsq, in0=mean, scalar=-1.0,
                        in1=mean, op0=MUL, op1=MUL)
                    var = small.tile([P, 1], F32, tag="var")
                    nc.vector.tensor_scalar(
                        out=var, in0=ssq, scalar1=1.0 / D, scalar2=nmsq,
                        op0=MUL, op1=ADD)
                    rstd = small.tile([P, 1], F32, tag="rstd")
                    nc.scalar.activation(
                        out=rstd, in_=var, func=SQRT, bias=epst)
                    nc.vector.reciprocal(out=rstd, in_=rstd)
                    nmr = small.tile([P, 1], F32, tag="nmr")
                    nc.vector.scalar_tensor_tensor(
                        out=nmr, in0=mean, scalar=-1.0,
                        in1=rstd, op0=MUL, op1=MUL)
                    xh = lnsc.tile([P, D], BF16, tag="xh")
                    nc.scalar.activation(
                        out=xh, in_=xs, func=IDENT, scale=rstd, bias=nmr)
                    eng = nc.vector if lt % 2 == 0 else nc.gpsimd
                    eng.tensor_mul(out=xh, in0=xh, in1=g1b)
                    eng.tensor_add(out=x1b[:, lt, :], in0=xh, in1=be1b)
                    nc.sync.dma_start_transpose(
                        out=x1T[:, lt, :, :], in_=x1b[:, lt, :])

            # ---------------- FFN ----------------
            with (
                tc.tile_pool(name="ffn_sb", bufs=1) as fsb,
                tc.tile_pool(name="w1p", bufs=3) as w1p,
                tc.tile_pool(name="otp", bufs=2) as otp,
            ):
                g2b = gbp.tile([P, D], F32, tag="g2")
                nc.gpsimd.dma_start(out=g2b, in_=bcast(g2, D))
                be2b = gbp.tile([P, D], F32, tag="be2")
                nc.gpsimd.dma_start(out=be2b, in_=bcast(be2, D))
                b2b = gbp.tile([P, D], F32, tag="b2")
                nc.gpsimd.dma_start(out=b2b, in_=bcast(b2, D))

                # all w2 stripes + all h^T tiles stay resident
                w2a = fsb.tile([P, FT, D], BF16)
                for j in range(FT):
                    nc.sync.dma_start(out=w2a[:, j, :], in_=w2s[j])
                hts = fsb.tile([P, FT, Lq], BF16)

                # FFN1: h^T[f, l] = relu(w1 x1^T + b1)
                with tc.tile_pool(name="hpp", bufs=4, space="PSUM") as hpp:
                    for ft in range(FT):
                        wt = w1p.tile([P, D], BF16, tag="w1")
                        nc.sync.dma_start(out=wt, in_=w1s[ft])
                        hp = [hpp.tile([P, SL], F32, name=f"hp{s}",
                                       tag=f"hp{s}")
                              for s in range(NSL)]
                        # first two f-chunks: finish slab 0 (query rows
                        # 0..511) before touching slab 1, so FFN1 starts as
                        # soon as LN1 of the first 4 row tiles lands
                        if ft < 2:
                            loop = [(s, dc) for s in range(NSL)
                                    for dc in range(DT)]
                        else:
                            loop = [(s, dc) for dc in range(DT)
                                    for s in range(NSL)]
                        for s, dc in loop:
                            nc.tensor.matmul(
                                hp[s],
                                wt[:, dc * P:(dc + 1) * P],
                                x1T[:, s * (LT // NSL):(s + 1) * (LT // NSL), dc, :],
                                start=(dc == 0), stop=(dc == DT - 1))
                        for s in range(NSL):
                            nc.scalar.activation(
                                out=hts[:, ft, s * SL:(s + 1) * SL],
                                in_=hp[s], func=RELU,
                                bias=b1s[:, ft:ft + 1])

                # FFN2: y[l, d] = sum_j (h^T_j)^T w2_j  (row-major output)
                with tc.tile_pool(name="ypp", bufs=2, space="PSUM") as ypp:
                    for lt in range(LT):
                        yp = ypp.tile([P, D], F32)
                        for j in range(FT):
                            for s in range(NSL):
                                nc.tensor.matmul(
                                    yp[:, s * SL:(s + 1) * SL],
                                    hts[:, j, lt * P:(lt + 1) * P],
                                    w2a[:, j, s * SL:(s + 1) * SL],
                                    start=(j == 0), stop=(j == FT - 1))
                        # residual 2 + b2 + LN2 -> out
                        xs = resp.tile([P, D], F32, tag="res")
                        nc.vector.scalar_tensor_tensor(
                            out=xs, in0=yp, scalar=1.0,
                            in1=x1b[:, lt, :], op0=MUL, op1=ADD)
                        nc.vector.tensor_add(out=xs, in0=xs, in1=b2b)
                        ot = otp.tile([P, D], F32, tag="ot")
                        _layer_norm(nc, small, ot, xs, g2b, be2b, epst, GS)
                        nc.sync.dma_start(
                            out=out[lt * P:(lt + 1) * P, :], in_=ot)

    nc.finalize()
    return nc


def _layer_norm(nc, small, out_ap, x_ap, gb, beb, epst, GS):
    """out = (x - mean(x)) * rsqrt(var(x) + eps) * g + be over free dim.
    x_ap is clobbered (normalized in place); out_ap gets the final value
    and may have a different dtype."""
    D = x_ap.shape[-1]
    ngr = D // GS
    st = small.tile([P, ngr, 6], F32, tag="bnst")
    xg = x_ap.rearrange("p (g k) -> p g k", k=GS)
    for g in range(ngr):
        nc.vector.bn_stats(out=st[:, g, :], in_=xg[:, g, :])
    mv = small.tile([P, 2], F32, tag="bnmv")
    nc.vector.bn_aggr(out=mv, in_=st)
    rstd = small.tile([P, 1], F32, tag="rstd")
    nc.scalar.activation(out=rstd, in_=mv[:, 1:2], func=SQRT, bias=epst)
    nc.vector.reciprocal(out=rstd, in_=rstd)
    nc.vector.tensor_scalar(
        out=x_ap, in0=x_ap, scalar1=mv[:, 0:1], scalar2=rstd,
        op0=SUB, op1=MUL)
    nc.vector.tensor_mul(out=x_ap, in0=x_ap, in1=gb)
    nc.vector.tensor_add(out=out_ap, in0=x_ap, in1=beb)


# ---------------------------------------------------------------------------
# host side
# ---------------------------------------------------------------------------

_PROG_CACHE = {}


def get_program(S=2048, D=1024, F=4096):
    key = (S, D, F)
    if key not in _PROG_CACHE:
        _PROG_CACHE[key] = build_program(S, D, F)
    return _PROG_CACHE[key]


def make_in_maps(x, w1, b1, w2, b2, g1, be1, g2, be2, n_cores=8):
    B, L, D = x.shape
    F = w1.shape[0]
    Lq = L // 2
    DT, FT = D // 128, F // 128
    import ml_dtypes
    # w1s[ft, p, dc*128+f] = w1[ft*128+f, dc*128+p]
    w1s = np.ascontiguousarray(
        w1.reshape(FT, 128, DT, 128).transpose(0, 3, 2, 1)
        .reshape(FT, 128, D)).astype(ml_dtypes.bfloat16)
    # w2s[j, p, d] = w2[d, j*128+p]
    w2s = np.ascontiguousarray(
        w2.T.reshape(FT, 128, D)).astype(ml_dtypes.bfloat16)
    common = dict(w1s=w1s, w2s=w2s, b1=b1, b2=b2,
                  g1h=np.asarray(g1, ml_dtypes.bfloat16),
                  be1h=np.asarray(be1, ml_dtypes.bfloat16),
                  g2=g2, be2=be2)
    in_maps = []
    for c in range(n_cores):
        b, half = c // 2, c % 2
        lo = half * Lq
        xq = x[b, lo:lo + Lq]
        xo = x[b, Lq - lo:2 * Lq - lo]
        xbl = np.ascontiguousarray(np.concatenate([xq, xo], axis=0))
        in_maps.append(dict(xb=xbl, xb16=xbl.astype(ml_dtypes.bfloat16),
                            **common))
    return in_maps


def kernel(x, w1, b1, w2, b2, g1, be1, g2, be2):
    from concourse.bass_utils import run_bass_kernel_spmd

    x = np.asarray(x, dtype=np.float32)
    B, L, D = x.shape
    F = w1.shape[0]
    Lq = L // 2
    n_cores = 2 * B
    nc = get_program(L, D, F)
    in_maps = make_in_maps(x, np.asarray(w1, np.float32), np.asarray(b1, np.float32),
                           np.asarray(w2, np.float32), np.asarray(b2, np.float32),
                           np.asarray(g1, np.float32), np.asarray(be1, np.float32),
                           np.asarray(g2, np.float32), np.asarray(be2, np.float32),
                           n_cores)
    res = run_bass_kernel_spmd(nc, in_maps, core_ids=list(range(n_cores)))
    outp = np.empty((B, L, D), dtype=np.float32)
    for c in range(n_cores):
        b, half = c // 2, c % 2
        outp[b, half * Lq:(half + 1) * Lq] = res.results[c]["out"]
    return outp


# revision 29
# speedup vs baseline: 1.0080x; 1.0080x over previous
"""Trainium2 Bass kernel for a dense transformer encoder layer.

Reference computation (per batch b):
    q = x.reshape(L, H, E)                       # H=16 heads, E=64
    scores = q @ q^T per head, scaled softmax    # A = softmax(s/8)
    new_x  = concat_h(A_h @ q_h)                 # [L, D]
    x1 = LN(x + new_x; g1, be1)
    y  = relu(x1 @ w1^T + b1) @ w2^T + b2
    out = LN(x1 + y; g2, be2)

Sharding: pure data parallel over (batch, seq-half): core c handles
batch c//2, query rows [(c%2)*1024, +1024).  Keys/values span the full
sequence of that batch, so every core gets the whole x[b] (queries
reordered first) and the full FFN weights.  No device collectives.

v2 design notes (all matmuls bf16, PE does ONLY matmuls):
  - x^T, U^T, x1^T are produced with DMA xbar transposes (16-bit dtype,
    src partition %16, free %128), not PE transposes.
  - scores are computed TRANSPOSED ([s, l]) so exp(scores^T) is the
    moving operand of the AV matmul; V carries a ones column so the
    softmax denominator rides along in row 64 of U^T (rows 65..79 pad
    to the xbar 16-row granularity with zero columns).
  - FFN weights are streamed from HBM once, as one [128, 1024] stripe
    per 128-row block (64 DMAs total instead of 1024 tile DMAs).
  - FFN1 accumulates over d-chunks with the stationary w1 tile reused
    across both 512-wide moving slabs; FFN2 uses h^T tiles as the
    stationary and w2 stripes as the moving operand, producing y
    ROW-major directly into PSUM (no output transpose at all).
"""

import numpy as np

import concourse.bass as bass
import concourse.tile as tile
from concourse import bacc
from concourse import mybir

F32 = mybir.dt.float32
BF16 = mybir.dt.bfloat16
EXP = mybir.ActivationFunctionType.Exp
RELU = mybir.ActivationFunctionType.Relu
SQRT = mybir.ActivationFunctionType.Sqrt
SQUARE = mybir.ActivationFunctionType.Square
IDENT = mybir.ActivationFunctionType.Identity
ADD = mybir.AluOpType.add
SUB = mybir.AluOpType.subtract
MUL = mybir.AluOpType.mult

LN_EPS = 1e-5
E = 64          # head dim
W = 80          # head dim + ones column + pad to xbar 16-row granularity
P = 128         # partitions


def build_program(S=2048, D=1024, F=4096):
    """Per-core program.  S = full seq len; queries are rows [0, Lq)."""
    H = D // E
    Lq = S // 2
    ST = S // P          # key tiles
    LT = Lq // P         # query row tiles
    DT = D // P          # d chunks
    FT = F // P          # f chunks
    NSL = 2
    SL = Lq // NSL       # moving slab width (512)
    GS = min(512, D)     # bn_stats subgroup size

    nc = bacc.Bacc("TRN2")

    xb = nc.dram_tensor("xb", [S, D], F32, kind="ExternalInput")
    xb16 = nc.dram_tensor("xb16", [S, D], BF16, kind="ExternalInput")
    w1s = nc.dram_tensor("w1s", [FT, P, D], BF16, kind="ExternalInput")
    w2s = nc.dram_tensor("w2s", [FT, P, D], BF16, kind="ExternalInput")
    b1 = nc.dram_tensor("b1", [F], F32, kind="ExternalInput")
    b2 = nc.dram_tensor("b2", [D], F32, kind="ExternalInput")
    g1h = nc.dram_tensor("g1h", [D], BF16, kind="ExternalInput")
    be1h = nc.dram_tensor("be1h", [D], BF16, kind="ExternalInput")
    g2 = nc.dram_tensor("g2", [D], F32, kind="ExternalInput")
    be2 = nc.dram_tensor("be2", [D], F32, kind="ExternalInput")
    out = nc.dram_tensor("out", [Lq, D], F32, kind="ExternalOutput")

    def bcast(dram_vec, n):
        a = dram_vec[:]
        return bass.AP(tensor=a.tensor, offset=a.offset, ap=[[0, P]] + a.ap)

    with tile.TileContext(nc) as tc:
        with (
            tc.tile_pool(name="persist", bufs=1) as persist,
            tc.tile_pool(name="small", bufs=6) as small,
            tc.tile_pool(name="gb", bufs=1) as gbp,
            tc.tile_pool(name="resp", bufs=2) as resp,
        ):
            b1s = persist.tile([P, FT], F32)
            nc.sync.dma_start(out=b1s, in_=b1[:].rearrange("(t p) -> p t", p=P))
            epst = persist.tile([P, 1], F32)
            nc.vector.memset(epst, LN_EPS)
            # x1 (post-LN1) in bf16: residual-2 source and FFN1 input
            x1b = persist.tile([P, LT, D], BF16)
            # x1^T: [p, lt, dc, j] = x1[lt*128+j, dc*128+p]
            x1T = persist.tile([P, LT, DT, P], BF16)

            # ---------------- attention ----------------
            with (
                tc.tile_pool(name="attn_sb", bufs=1) as asb,
                tc.tile_pool(name="xrp", bufs=2) as xrp,
                tc.tile_pool(name="etp", bufs=4) as etp,
                tc.tile_pool(name="utsp", bufs=2) as utsp,
                tc.tile_pool(name="usp", bufs=3) as usp,
                tc.tile_pool(name="recp", bufs=4) as recp,
                tc.tile_pool(name="lnsc", bufs=3) as lnsc,
            ):
                # attention output, bf16 (residual add upcasts later)
                new_x = asb.tile([P, LT, D], BF16)
                # x^T tiles: [P, DT, S]; d-chunk t holds heads 2t, 2t+1.
                # Interleave the transposes with the vaug row loads so the
                # first heads' operands land as early as possible.
                xT = asb.tile([P, DT, S], BF16)
                vaug = asb.tile([P, ST, H, W], BF16)
                # query rows (bf16) kept for the LN1 residual
                xrows = asb.tile([P, LT, D], BF16)
                nc.gpsimd.memset(vaug[:, :, :, E:W], 0.0)
                nc.gpsimd.memset(vaug[:, :, :, E:E + 1], 1.0)
                nc.sync.dma_start_transpose(
                    out=xT[:, 0, :], in_=xb16[:, 0:P])
                for u in range(ST):
                    if u < LT:
                        xr = xrows[:, u, :]
                    else:
                        xr = xrp.tile([P, D], BF16, tag="xr")
                    nc.sync.dma_start(out=xr, in_=xb16[u * P:(u + 1) * P, :])
                    nc.vector.tensor_copy(
                        out=vaug[:, u, :, 0:E],
                        in_=xr.rearrange("p (h e) -> p h e", e=E))
                    if u % 2 == 1 and 1 + u // 2 < DT:
                        t = 1 + u // 2
                        nc.sync.dma_start_transpose(
                            out=xT[:, t, :], in_=xb16[:, t * P:(t + 1) * P])

                g1b = gbp.tile([P, D], BF16, tag="g1")
                nc.gpsimd.dma_start(out=g1b, in_=bcast(g1h, D))
                be1b = gbp.tile([P, D], BF16, tag="be1")
                nc.gpsimd.dma_start(out=be1b, in_=bcast(be1h, D))

                def head_epilogue(h, ut):
                    uts = utsp.tile([W, Lq], BF16, name="uts", tag="uts")
                    nc.vector.tensor_copy(out=uts, in_=ut)
                    # U: [p, lt, w] = U^T[w, lt*128+p]
                    us = usp.tile([P, LT, W], BF16, name="us", tag="us")
                    nc.sync.dma_start_transpose(out=us, in_=uts)
                    rec = recp.tile([P, LT], F32, name="rec", tag="rec")
                    nc.vector.reciprocal(out=rec, in_=us[:, :, E])
                    for lt in range(LT):
                        nc.vector.tensor_scalar_mul(
                            out=new_x[:, lt, h * E:(h + 1) * E],
                            in0=us[:, lt, 0:E],
                            scalar1=rec[:, lt:lt + 1])

                with (
                    tc.tile_pool(name="scp", bufs=2, space="PSUM") as scp,
                    tc.tile_pool(name="utp", bufs=2, space="PSUM") as utp,
                ):
                    # AV lags TWO chunks behind scores/exp and flows across
                    # head boundaries, so neither the PE nor ACT ever stalls
                    # on the other inside the attention loop.
                    pend = []

                    def emit_one():
                        et_p, u_p, ut_p, h_p = pend.pop(0)
                        for s in range(NSL):
                            nc.tensor.matmul(
                                ut_p[:, s * SL:(s + 1) * SL],
                                vaug[:, u_p, h_p, :],
                                et_p[:, s * SL:(s + 1) * SL],
                                start=(u_p == 0), stop=(u_p == ST - 1))
                        if u_p == ST - 1:
                            head_epilogue(h_p, ut_p)

                    for h in range(H):
                        t, ro = h // 2, (h % 2) * E
                        ut = utp.tile([W, Lq], F32, name="ut", tag="ut")
                        for u in range(ST):
                            sc = scp.tile([P, Lq], F32, name="sc", tag="sc")
                            et = etp.tile([P, Lq], BF16, name="et", tag="et")
                            for s in range(NSL):
                                nc.tensor.matmul(
                                    sc[:, s * SL:(s + 1) * SL],
                                    xT[ro:ro + E, t, u * P:(u + 1) * P],
                                    xT[ro:ro + E, t, s * SL:(s + 1) * SL],
                                    start=True, stop=True)
                            if len(pend) >= 2:
                                emit_one()
                            nc.scalar.activation(
                                out=et, in_=sc, func=EXP, scale=1.0 / 8.0)
                            pend.append((et, u, ut, h))
                    while pend:
                        emit_one()

                # residual 1 + LN1 -> x1b (bf16) and x1T (dma transpose).
                # Work is spread over DVE / ACT / GpSimd and pipelined
                # across row tiles to shorten the attention->FFN boundary.
                for lt in range(LT):
                    xs = resp.tile([P, D], F32, tag="res")
                    rs = small.tile([P, 1], F32, tag="rs")
                    nc.vector.scalar_tensor_tensor(
                        out=xs, in0=new_x[:, lt, :], scalar=1.0,
                        in1=xrows[:, lt, :], op0=MUL, op1=ADD, accum_out=rs)
                    mean = small.tile([P, 1], F32, tag="mean")
                    nc.vector.tensor_scalar_mul(
                        out=mean, in0=rs, scalar1=1.0 / D)
                    sq = lnsc.tile([P, D], F32, tag="sq", bufs=1)
                    ssq = small.tile([P, 1], F32, tag="ssq")
                    nc.scalar.activation(
                        out=sq, in_=xs, func=SQUARE, accum_out=ssq)
                    nmsq = small.tile([P, 1], F32, tag="nmsq")
                    # var = ssq/D - mean^2  (+eps under the sqrt)
                    nc.vector.scalar_tensor_tensor(
                        out=nmsq, in0=mean, scalar=-1.0,
                        in1=mean, op0=MUL, op1=MUL)
                    var = small.tile([P, 1], F32, tag="var")
                    nc.vector.tensor_scalar(
                        out=var, in0=ssq, scalar1=1.0 / D, scalar2=nmsq,
                        op0=MUL, op1=ADD)
                    rstd = small.tile([P, 1], F32, tag="rstd")
                    nc.scalar.activation(
                        out=rstd, in_=var, func=SQRT, bias=epst)
                    nc.vector.reciprocal(out=rstd, in_=rstd)
                    nmr = small.tile([P, 1], F32, tag="nmr")
                    nc.vector.scalar_tensor_tensor(
                        out=nmr, in0=mean, scalar=-1.0,
                        in1=rstd, op0=MUL, op1=MUL)
                    xh = lnsc.tile([P, D], BF16, tag="xh")
                    nc.scalar.activation(
                        out=xh, in_=xs, func=IDENT, scale=rstd, bias=nmr)
                    eng = nc.vector if lt % 2 == 0 else nc.gpsimd
                    eng.tensor_mul(out=xh, in0=xh, in1=g1b)
                    eng.tensor_add(out=x1b[:, lt, :], in0=xh, in1=be1b)
                    nc.sync.dma_start_transpose(
                        out=x1T[:, lt, :, :], in_=x1b[:, lt, :])

            # ---------------- FFN ----------------
            with (
                tc.tile_pool(name="ffn_sb", bufs=1) as fsb,
                tc.tile_pool(name="w1p", bufs=3) as w1p,
                tc.tile_pool(name="otp", bufs=2) as otp,
            ):
                g2b = gbp.tile([P, D], F32, tag="g2")
                nc.gpsimd.dma_start(out=g2b, in_=bcast(g2, D))
                be2b = gbp.tile([P, D], F32, tag="be2")
                nc.gpsimd.dma_start(out=be2b, in_=bcast(be2, D))
                b2b = gbp.tile([P, D], F32, tag="b2")
                nc.gpsimd.dma_start(out=b2b, in_=bcast(b2, D))

                # all w2 stripes + all h^T tiles stay resident
                w2a = fsb.tile([P, FT, D], BF16)
                for j in range(FT):
                    nc.sync.dma_start(out=w2a[:, j, :], in_=w2s[j])
                hts = fsb.tile([P, FT, Lq], BF16)

                # FFN1: h^T[f, l] = relu(w1 x1^T + b1)
                with tc.tile_pool(name="hpp", bufs=4, space="PSUM") as hpp:
                    for ft in range(FT):
                        wt = w1p.tile([P, D], BF16, tag="w1")
                        nc.sync.dma_start(out=wt, in_=w1s[ft])
                        hp = [hpp.tile([P, SL], F32, name=f"hp{s}",
                                       tag=f"hp{s}")
                              for s in range(NSL)]
                        # first two f-chunks: finish slab 0 (query rows
                        # 0..511) before touching slab 1, so FFN1 starts as
                        # soon as LN1 of the first 4 row tiles lands
                        if ft < 2:
                            loop = [(s, dc) for s in range(NSL)
                                    for dc in range(DT)]
                        else:
                            loop = [(s, dc) for dc in range(DT)
                                    for s in range(NSL)]
                        for s, dc in loop:
                            nc.tensor.matmul(
                                hp[s],
                                wt[:, dc * P:(dc + 1) * P],
                                x1T[:, s * (LT // NSL):(s + 1) * (LT // NSL), dc, :],
                                start=(dc == 0), stop=(dc == DT - 1))
                        for s in range(NSL):
                            nc.scalar.activation(
                                out=hts[:, ft, s * SL:(s + 1) * SL],
                                in_=hp[s], func=RELU,
                                bias=b1s[:, ft:ft + 1])

                # FFN2: y[l, d] = sum_j (h^T_j)^T w2_j  (row-major output)
                with tc.tile_pool(name="ypp", bufs=2, space="PSUM") as ypp:
                    for lt in range(LT):
                        yp = ypp.tile([P, D], F32)
                        for j in range(FT):
                            for s in range(NSL):
                                nc.tensor.matmul(
                                    yp[:, s * SL:(s + 1) * SL],
                                    hts[:, j, lt * P:(lt + 1) * P],
                                    w2a[:, j, s * SL:(s + 1) * SL],
                                    start=(j == 0), stop=(j == FT - 1))
                        # residual 2 + b2 + LN2 -> out
                        xs = resp.tile([P, D], F32, tag="res")
                        nc.vector.scalar_tensor_tensor(
                            out=xs, in0=yp, scalar=1.0,
                            in1=x1b[:, lt, :], op0=MUL, op1=ADD)
                        nc.vector.tensor_add(out=xs, in0=xs, in1=b2b)
                        ot = otp.tile([P, D], F32, tag="ot")
                        _layer_norm(nc, small, ot, xs, g2b, be2b, epst, GS)
                        nc.sync.dma_start(
                            out=out[lt * P:(lt + 1) * P, :], in_=ot)

    nc.finalize()
    return nc


def _layer_norm(nc, small, out_ap, x_ap, gb, beb, epst, GS):
    """out = (x - mean(x)) * rsqrt(var(x) + eps) * g + be over free dim.
    x_ap is clobbered (normalized in place); out_ap gets the final value
    and may have a different dtype."""
    D = x_ap.shape[-1]
    ngr = D // GS
    st = small.tile([P, ngr, 6], F32, tag="bnst")
    xg = x_ap.rearrange("p (g k) -> p g k", k=GS)
    for g in range(ngr):
        nc.vector.bn_stats(out=st[:, g, :], in_=xg[:, g, :])
    mv = small.tile([P, 2], F32, tag="bnmv")
    nc.vector.bn_aggr(out=mv, in_=st)
    rstd = small.tile([P, 1], F32, tag="rstd")
    nc.scalar.activation(out=rstd, in_=mv[:, 1:2], func=SQRT, bias=epst)
    nc.vector.reciprocal(out=rstd, in_=rstd)
    nc.vector.tensor_scalar(
        out=x_ap, in0=x_ap, scalar1=mv[:, 0:1], scalar2=rstd,
        op0=SUB, op1=MUL)
    nc.vector.tensor_mul(out=x_ap, in0=x_ap, in1=gb)
    nc.vector.tensor_add(out=out_ap, in0=x_ap, in1=beb)


# ---------------------------------------------------------------------------
# host side
# ---------------------------------------------------------------------------

_PROG_CACHE = {}


def get_program(S=2048, D=1024, F=4096):
    key = (S, D, F)
    if key not in _PROG_CACHE:
        _PROG_CACHE[key] = build_program(S, D, F)
    return _PROG_CACHE[key]


def make_in_maps(x, w1, b1, w2, b2, g1, be1, g2, be2, n_cores=8):
    B, L, D = x.shape
    F = w1.shape[0]
    Lq = L // 2
    DT, FT = D // 128, F // 128
    import ml_dtypes
    # w1s[ft, p, dc*128+f] = w1[ft*128+f, dc*128+p]
    w1s = np.ascontiguousarray(
        w1.reshape(FT, 128, DT, 128).transpose(0, 3, 2, 1)
        .reshape(FT, 128, D)).astype(ml_dtypes.bfloat16)
    # w2s[j, p, d] = w2[d, j*128+p]
    w2s = np.ascontiguousarray(
        w2.T.reshape(FT, 128, D)).astype(ml_dtypes.bfloat16)
    common = dict(w1s=w1s, w2s=w2s, b1=b1, b2=b2,
                  g1h=np.asarray(g1, ml_dtypes.bfloat16),
                  be1h=np.asarray(be1, ml_dtypes.bfloat16),
                  g2=g2, be2=be2)
    in_maps = []
    for c in range(n_cores):
        b, half = c // 2, c % 2
        lo = half * Lq
        xq = x[b, lo:lo + Lq]
        xo = x[b, Lq - lo:2 * Lq - lo]
        xbl = np.ascontiguousarray(np.concatenate([xq, xo], axis=0))
        in_maps.append(dict(xb=xbl, xb16=xbl.astype(ml_dtypes.bfloat16),
                            **common))
    return in_maps


def kernel(x, w1, b1, w2, b2, g1, be1, g2, be2):
    from concourse.bass_utils import run_bass_kernel_spmd

    x = np.asarray(x, dtype=np.float32)
    B, L, D = x.shape
    F = w1.shape[0]
    Lq = L // 2
    n_cores = 2 * B
    nc = get_program(L, D, F)
    in_maps = make_in_maps(x, np.asarray(w1, np.float32), np.asarray(b1, np.float32),
                           np.asarray(w2, np.float32), np.asarray(b2, np.float32),
                           np.asarray(g1, np.float32), np.asarray(be1, np.float32),
                           np.asarray(g2, np.float32), np.asarray(be2, np.float32),
                           n_cores)
    res = run_bass_kernel_spmd(nc, in_maps, core_ids=list(range(n_cores)))
    outp = np.empty((B, L, D), dtype=np.float32)
    for c in range(n_cores):
        b, half = c // 2, c % 2
        outp[b, half * Lq:(half + 1) * Lq] = res.results[c]["out"]
    return outp


# revision 30
# speedup vs baseline: 1.1992x; 1.1897x over previous
"""Trainium2 Bass kernel for a dense transformer encoder layer.

Reference computation (per batch b):
    q = x.reshape(L, H, E)                       # H=16 heads, E=64
    scores = q @ q^T per head, scaled softmax    # A = softmax(s/8)
    new_x  = concat_h(A_h @ q_h)                 # [L, D]
    x1 = LN(x + new_x; g1, be1)
    y  = relu(x1 @ w1^T + b1) @ w2^T + b2
    out = LN(x1 + y; g2, be2)

Sharding: pure data parallel over (batch, seq-half): core c handles
batch c//2, query rows [(c%2)*1024, +1024).  Keys/values span the full
sequence of that batch, so every core gets the whole x[b] (queries
reordered first) and the full FFN weights.  No device collectives.

v2 design notes (all matmuls bf16, PE does ONLY matmuls):
  - x^T, U^T, x1^T are produced with DMA xbar transposes (16-bit dtype,
    src partition %16, free %128), not PE transposes.
  - scores are computed TRANSPOSED ([s, l]) so exp(scores^T) is the
    moving operand of the AV matmul; V carries a ones column so the
    softmax denominator rides along in row 64 of U^T (rows 65..79 pad
    to the xbar 16-row granularity with zero columns).
  - FFN weights are streamed from HBM once, as one [128, 1024] stripe
    per 128-row block (64 DMAs total instead of 1024 tile DMAs).
  - FFN1 accumulates over d-chunks with the stationary w1 tile reused
    across both 512-wide moving slabs; FFN2 uses h^T tiles as the
    stationary and w2 stripes as the moving operand, producing y
    ROW-major directly into PSUM (no output transpose at all).
"""

import numpy as np

import concourse.bass as bass
import concourse.tile as tile
from concourse import bacc
from concourse import mybir

F32 = mybir.dt.float32
BF16 = mybir.dt.bfloat16
EXP = mybir.ActivationFunctionType.Exp
RELU = mybir.ActivationFunctionType.Relu
SQRT = mybir.ActivationFunctionType.Sqrt
SQUARE = mybir.ActivationFunctionType.Square
IDENT = mybir.ActivationFunctionType.Identity
ADD = mybir.AluOpType.add
SUB = mybir.AluOpType.subtract
MUL = mybir.AluOpType.mult

LN_EPS = 1e-5
E = 64          # head dim
W = 80          # head dim + ones column + pad to xbar 16-row granularity
P = 128         # partitions


def build_program(S=2048, D=1024, F=4096):
    """Per-core program.  S = full seq len; queries are rows [0, Lq)."""
    H = D // E
    Lq = S // 2
    ST = S // P          # key tiles
    LT = Lq // P         # query row tiles
    DT = D // P          # d chunks
    FT = F // P          # f chunks
    NSL = 2
    SL = Lq // NSL       # moving slab width (512)
    GS = min(512, D)     # bn_stats subgroup size

    nc = bacc.Bacc("TRN2")

    xb = nc.dram_tensor("xb", [S, D], F32, kind="ExternalInput")
    xb16 = nc.dram_tensor("xb16", [S, D], BF16, kind="ExternalInput")
    w1s = nc.dram_tensor("w1s", [FT, P, D], BF16, kind="ExternalInput")
    w2s = nc.dram_tensor("w2s", [FT, P, D], BF16, kind="ExternalInput")
    b1 = nc.dram_tensor("b1", [F], F32, kind="ExternalInput")
    b2 = nc.dram_tensor("b2", [D], F32, kind="ExternalInput")
    g1h = nc.dram_tensor("g1h", [D], BF16, kind="ExternalInput")
    be1h = nc.dram_tensor("be1h", [D], BF16, kind="ExternalInput")
    g2 = nc.dram_tensor("g2", [D], F32, kind="ExternalInput")
    be2 = nc.dram_tensor("be2", [D], F32, kind="ExternalInput")
    out = nc.dram_tensor("out", [Lq, D], F32, kind="ExternalOutput")

    def bcast(dram_vec, n):
        a = dram_vec[:]
        return bass.AP(tensor=a.tensor, offset=a.offset, ap=[[0, P]] + a.ap)

    with tile.TileContext(nc) as tc:
        with (
            tc.tile_pool(name="persist", bufs=1) as persist,
            tc.tile_pool(name="small", bufs=6) as small,
            tc.tile_pool(name="gb", bufs=1) as gbp,
            tc.tile_pool(name="resp", bufs=2) as resp,
        ):
            b1s = persist.tile([P, FT], F32)
            nc.sync.dma_start(out=b1s, in_=b1[:].rearrange("(t p) -> p t", p=P))
            epst = persist.tile([P, 1], F32)
            nc.vector.memset(epst, LN_EPS)
            # x1 (post-LN1) in bf16: residual-2 source and FFN1 input
            x1b = persist.tile([P, LT, D], BF16)
            # x1^T: [p, lt, dc, j] = x1[lt*128+j, dc*128+p]
            x1T = persist.tile([P, LT, DT, P], BF16)

            # ---------------- attention ----------------
            with (
                tc.tile_pool(name="attn_sb", bufs=1) as asb,
                tc.tile_pool(name="xrp", bufs=2) as xrp,
                tc.tile_pool(name="etp", bufs=4) as etp,
                tc.tile_pool(name="utsp", bufs=2) as utsp,
                tc.tile_pool(name="usp", bufs=3) as usp,
                tc.tile_pool(name="recp", bufs=4) as recp,
                tc.tile_pool(name="lnsc", bufs=3) as lnsc,
            ):
                # attention output, bf16 (residual add upcasts later)
                new_x = asb.tile([P, LT, D], BF16)
                # x^T tiles: [P, DT, S]; d-chunk t holds heads 2t, 2t+1.
                # Interleave the transposes with the vaug row loads so the
                # first heads' operands land as early as possible.
                xT = asb.tile([P, DT, S], BF16)
                vaug = asb.tile([P, ST, H, W], BF16)
                # query rows (bf16) kept for the LN1 residual
                xrows = asb.tile([P, LT, D], BF16)
                nc.gpsimd.memset(vaug[:, :, :, E:W], 0.0)
                nc.gpsimd.memset(vaug[:, :, :, E:E + 1], 1.0)
                nc.sync.dma_start_transpose(
                    out=xT[:, 0, :], in_=xb16[:, 0:P])
                for u in range(ST):
                    if u < LT:
                        xr = xrows[:, u, :]
                    else:
                        xr = xrp.tile([P, D], BF16, tag="xr")
                    nc.sync.dma_start(out=xr, in_=xb16[u * P:(u + 1) * P, :])
                    nc.vector.tensor_copy(
                        out=vaug[:, u, :, 0:E],
                        in_=xr.rearrange("p (h e) -> p h e", e=E))
                    if u % 2 == 1 and 1 + u // 2 < DT:
                        t = 1 + u // 2
                        nc.sync.dma_start_transpose(
                            out=xT[:, t, :], in_=xb16[:, t * P:(t + 1) * P])

                g1b = gbp.tile([P, D], BF16, tag="g1")
                nc.gpsimd.dma_start(out=g1b, in_=bcast(g1h, D))
                be1b = gbp.tile([P, D], BF16, tag="be1")
                nc.gpsimd.dma_start(out=be1b, in_=bcast(be1h, D))

                def head_epilogue(h, ut):
                    uts = utsp.tile([W, Lq], BF16, name="uts", tag="uts")
                    nc.vector.tensor_copy(out=uts, in_=ut)
                    # U: [p, lt, w] = U^T[w, lt*128+p]
                    us = usp.tile([P, LT, W], BF16, name="us", tag="us")
                    nc.sync.dma_start_transpose(out=us, in_=uts)
                    rec = recp.tile([P, LT], F32, name="rec", tag="rec")
                    nc.vector.reciprocal(out=rec, in_=us[:, :, E])
                    for lt in range(LT):
                        nc.vector.tensor_scalar_mul(
                            out=new_x[:, lt, h * E:(h + 1) * E],
                            in0=us[:, lt, 0:E],
                            scalar1=rec[:, lt:lt + 1])

                with (
                    tc.tile_pool(name="scp", bufs=2, space="PSUM") as scp,
                    tc.tile_pool(name="utp", bufs=2, space="PSUM") as utp,
                ):
                    for h in range(H):
                        t, ro = h // 2, (h % 2) * E
                        ut = utp.tile([W, Lq], F32, name="ut", tag="ut")

                        def emit_ut(et_u, ut=ut, h=h):
                            et_p, u_p = et_u
                            for s in range(NSL):
                                nc.tensor.matmul(
                                    ut[:, s * SL:(s + 1) * SL],
                                    vaug[:, u_p, h, :],
                                    et_p[:, s * SL:(s + 1) * SL],
                                    start=(u_p == 0), stop=(u_p == ST - 1))

                        # software pipeline: AV lags TWO chunks so every PE
                        # instruction's exp input is long done -> the PE
                        # issues back-to-back with no semaphore stalls
                        pend = []
                        for u in range(ST):
                            sc = scp.tile([P, Lq], F32, name="sc", tag="sc")
                            et = etp.tile([P, Lq], BF16, name="et", tag="et")
                            for s in range(NSL):
                                nc.tensor.matmul(
                                    sc[:, s * SL:(s + 1) * SL],
                                    xT[ro:ro + E, t, u * P:(u + 1) * P],
                                    xT[ro:ro + E, t, s * SL:(s + 1) * SL],
                                    start=True, stop=True)
                            if len(pend) >= 2:
                                emit_ut(pend.pop(0))
                            nc.scalar.activation(
                                out=et, in_=sc, func=EXP, scale=1.0 / 8.0)
                            pend.append((et, u))
                        for p_ in pend:
                            emit_ut(p_)
                        head_epilogue(h, ut)

                # residual 1 + LN1 -> x1b (bf16) and x1T (dma transpose).
                # Work is spread over DVE / ACT / GpSimd and pipelined
                # across row tiles to shorten the attention->FFN boundary.
                for lt in range(LT):
                    xs = resp.tile([P, D], F32, tag="res")
                    rs = small.tile([P, 1], F32, tag="rs")
                    nc.vector.scalar_tensor_tensor(
                        out=xs, in0=new_x[:, lt, :], scalar=1.0,
                        in1=xrows[:, lt, :], op0=MUL, op1=ADD, accum_out=rs)
                    mean = small.tile([P, 1], F32, tag="mean")
                    nc.vector.tensor_scalar_mul(
                        out=mean, in0=rs, scalar1=1.0 / D)
                    sq = lnsc.tile([P, D], F32, tag="sq", bufs=1)
                    ssq = small.tile([P, 1], F32, tag="ssq")
                    nc.scalar.activation(
                        out=sq, in_=xs, func=SQUARE, accum_out=ssq)
                    nmsq = small.tile([P, 1], F32, tag="nmsq")
                    # var = ssq/D - mean^2  (+eps under the sqrt)
                    nc.vector.scalar_tensor_tensor(
                        out=nmsq, in0=mean, scalar=-1.0,
                        in1=mean, op0=MUL, op1=MUL)
                    var = small.tile([P, 1], F32, tag="var")
                    nc.vector.tensor_scalar(
                        out=var, in0=ssq, scalar1=1.0 / D, scalar2=nmsq,
                        op0=MUL, op1=ADD)
                    rstd = small.tile([P, 1], F32, tag="rstd")
                    nc.scalar.activation(
                        out=rstd, in_=var, func=SQRT, bias=epst)
                    nc.vector.reciprocal(out=rstd, in_=rstd)
                    nmr = small.tile([P, 1], F32, tag="nmr")
                    nc.vector.scalar_tensor_tensor(
                        out=nmr, in0=mean, scalar=-1.0,
                        in1=rstd, op0=MUL, op1=MUL)
                    xh = lnsc.tile([P, D], BF16, tag="xh")
                    nc.scalar.activation(
                        out=xh, in_=xs, func=IDENT, scale=rstd, bias=nmr)
                    eng = nc.vector if lt % 2 == 0 else nc.gpsimd
                    eng.tensor_mul(out=xh, in0=xh, in1=g1b)
                    eng.tensor_add(out=x1b[:, lt, :], in0=xh, in1=be1b)
                    nc.sync.dma_start_transpose(
                        out=x1T[:, lt, :, :], in_=x1b[:, lt, :])

            # ---------------- FFN ----------------
            with (
                tc.tile_pool(name="ffn_sb", bufs=1) as fsb,
                tc.tile_pool(name="w1p", bufs=3) as w1p,
                tc.tile_pool(name="otp", bufs=2) as otp,
            ):
                g2b = gbp.tile([P, D], F32, tag="g2")
                nc.gpsimd.dma_start(out=g2b, in_=bcast(g2, D))
                be2b = gbp.tile([P, D], F32, tag="be2")
                nc.gpsimd.dma_start(out=be2b, in_=bcast(be2, D))
                b2b = gbp.tile([P, D], F32, tag="b2")
                nc.gpsimd.dma_start(out=b2b, in_=bcast(b2, D))

                # all w2 stripes + all h^T tiles stay resident
                w2a = fsb.tile([P, FT, D], BF16)
                for j in range(FT):
                    nc.sync.dma_start(out=w2a[:, j, :], in_=w2s[j])
                hts = fsb.tile([P, FT, Lq], BF16)

                # FFN1: h^T[f, l] = relu(w1 x1^T + b1)
                with tc.tile_pool(name="hpp", bufs=4, space="PSUM") as hpp:
                    for ft in range(FT):
                        wt = w1p.tile([P, D], BF16, tag="w1")
                        nc.sync.dma_start(out=wt, in_=w1s[ft])
                        hp = [hpp.tile([P, SL], F32, name=f"hp{s}",
                                       tag=f"hp{s}")
                              for s in range(NSL)]
                        # first two f-chunks: finish slab 0 (query rows
                        # 0..511) before touching slab 1, so FFN1 starts as
                        # soon as LN1 of the first 4 row tiles lands
                        if ft < 2:
                            loop = [(s, dc) for s in range(NSL)
                                    for dc in range(DT)]
                        else:
                            loop = [(s, dc) for dc in range(DT)
                                    for s in range(NSL)]
                        for s, dc in loop:
                            nc.tensor.matmul(
                                hp[s],
                                wt[:, dc * P:(dc + 1) * P],
                                x1T[:, s * (LT // NSL):(s + 1) * (LT // NSL), dc, :],
                                start=(dc == 0), stop=(dc == DT - 1))
                        for s in range(NSL):
                            nc.scalar.activation(
                                out=hts[:, ft, s * SL:(s + 1) * SL],
                                in_=hp[s], func=RELU,
                                bias=b1s[:, ft:ft + 1])

                # FFN2: y[l, d] = sum_j (h^T_j)^T w2_j  (row-major output)
                with tc.tile_pool(name="ypp", bufs=2, space="PSUM") as ypp:
                    for lt in range(LT):
                        yp = ypp.tile([P, D], F32)
                        for j in range(FT):
                            for s in range(NSL):
                                nc.tensor.matmul(
                                    yp[:, s * SL:(s + 1) * SL],
                                    hts[:, j, lt * P:(lt + 1) * P],
                                    w2a[:, j, s * SL:(s + 1) * SL],
                                    start=(j == 0), stop=(j == FT - 1))
                        # residual 2 + b2 + LN2 -> out
                        xs = resp.tile([P, D], F32, tag="res")
                        nc.vector.scalar_tensor_tensor(
                            out=xs, in0=yp, scalar=1.0,
                            in1=x1b[:, lt, :], op0=MUL, op1=ADD)
                        nc.vector.tensor_add(out=xs, in0=xs, in1=b2b)
                        ot = otp.tile([P, D], F32, tag="ot")
                        _layer_norm(nc, small, ot, xs, g2b, be2b, epst, GS)
                        nc.sync.dma_start(
                            out=out[lt * P:(lt + 1) * P, :], in_=ot)

    nc.finalize()
    return nc


def _layer_norm(nc, small, out_ap, x_ap, gb, beb, epst, GS):
    """out = (x - mean(x)) * rsqrt(var(x) + eps) * g + be over free dim.
    x_ap is clobbered (normalized in place); out_ap gets the final value
    and may have a different dtype."""
    D = x_ap.shape[-1]
    ngr = D // GS
    st = small.tile([P, ngr, 6], F32, tag="bnst")
    xg = x_ap.rearrange("p (g k) -> p g k", k=GS)
    for g in range(ngr):
        nc.vector.bn_stats(out=st[:, g, :], in_=xg[:, g, :])
    mv = small.tile([P, 2], F32, tag="bnmv")
    nc.vector.bn_aggr(out=mv, in_=st)
    rstd = small.tile([P, 1], F32, tag="rstd")
    nc.scalar.activation(out=rstd, in_=mv[:, 1:2], func=SQRT, bias=epst)
    nc.vector.reciprocal(out=rstd, in_=rstd)
    nc.vector.tensor_scalar(
        out=x_ap, in0=x_ap, scalar1=mv[:, 0:1], scalar2=rstd,
        op0=SUB, op1=MUL)
    nc.vector.tensor_mul(out=x_ap, in0=x_ap, in1=gb)
    nc.vector.tensor_add(out=out_ap, in0=x_ap, in1=beb)


# ---------------------------------------------------------------------------
# host side
# ---------------------------------------------------------------------------

_PROG_CACHE = {}


def get_program(S=2048, D=1024, F=4096):
    key = (S, D, F)
    if key not in _PROG_CACHE:
        _PROG_CACHE[key] = build_program(S, D, F)
    return _PROG_CACHE[key]


def make_in_maps(x, w1, b1, w2, b2, g1, be1, g2, be2, n_cores=8):
    B, L, D = x.shape
    F = w1.shape[0]
    Lq = L // 2
    DT, FT = D // 128, F // 128
    import ml_dtypes
    # w1s[ft, p, dc*128+f] = w1[ft*128+f, dc*128+p]
    w1s = np.ascontiguousarray(
        w1.reshape(FT, 128, DT, 128).transpose(0, 3, 2, 1)
        .reshape(FT, 128, D)).astype(ml_dtypes.bfloat16)
    # w2s[j, p, d] = w2[d, j*128+p]
    w2s = np.ascontiguousarray(
        w2.T.reshape(FT, 128, D)).astype(ml_dtypes.bfloat16)
    common = dict(w1s=w1s, w2s=w2s, b1=b1, b2=b2,
                  g1h=np.asarray(g1, ml_dtypes.bfloat16),
                  be1h=np.asarray(be1, ml_dtypes.bfloat16),
                  g2=g2, be2=be2)
    in_maps = []
    for c in range(n_cores):
        b, half = c // 2, c % 2
        lo = half * Lq
        xq = x[b, lo:lo + Lq]
        xo = x[b, Lq - lo:2 * Lq - lo]
        xbl = np.ascontiguousarray(np.concatenate([xq, xo], axis=0))
        in_maps.append(dict(xb=xbl, xb16=xbl.astype(ml_dtypes.bfloat16),
                            **common))
    return in_maps


def kernel(x, w1, b1, w2, b2, g1, be1, g2, be2):
    from concourse.bass_utils import run_bass_kernel_spmd

    x = np.asarray(x, dtype=np.float32)
    B, L, D = x.shape
    F = w1.shape[0]
    Lq = L // 2
    n_cores = 2 * B
    nc = get_program(L, D, F)
    in_maps = make_in_maps(x, np.asarray(w1, np.float32), np.asarray(b1, np.float32),
                           np.asarray(w2, np.float32), np.asarray(b2, np.float32),
                           np.asarray(g1, np.float32), np.asarray(be1, np.float32),
                           np.asarray(g2, np.float32), np.asarray(be2, np.float32),
                           n_cores)
    res = run_bass_kernel_spmd(nc, in_maps, core_ids=list(range(n_cores)))
    outp = np.empty((B, L, D), dtype=np.float32)
    for c in range(n_cores):
        b, half = c // 2, c % 2
        outp[b, half * Lq:(half + 1) * Lq] = res.results[c]["out"]
    return outp


# revision 32
# speedup vs baseline: 1.2535x; 1.0452x over previous
"""Trainium2 Bass kernel for a dense transformer encoder layer.

Reference computation (per batch b):
    q = x.reshape(L, H, E)                       # H=16 heads, E=64
    scores = q @ q^T per head, scaled softmax    # A = softmax(s/8)
    new_x  = concat_h(A_h @ q_h)                 # [L, D]
    x1 = LN(x + new_x; g1, be1)
    y  = relu(x1 @ w1^T + b1) @ w2^T + b2
    out = LN(x1 + y; g2, be2)

Sharding: pure data parallel over (batch, seq-half): core c handles
batch c//2, query rows [(c%2)*1024, +1024).  Keys/values span the full
sequence of that batch, so every core gets the whole x[b] (queries
reordered first) and the full FFN weights.  No device collectives.

v2 design notes (all matmuls bf16, PE does ONLY matmuls):
  - x^T, U^T, x1^T are produced with DMA xbar transposes (16-bit dtype,
    src partition %16, free %128), not PE transposes.
  - scores are computed TRANSPOSED ([s, l]) so exp(scores^T) is the
    moving operand of the AV matmul; V carries a ones column so the
    softmax denominator rides along in row 64 of U^T (rows 65..79 pad
    to the xbar 16-row granularity with zero columns).
  - FFN weights are streamed from HBM once, as one [128, 1024] stripe
    per 128-row block (64 DMAs total instead of 1024 tile DMAs).
  - FFN1 accumulates over d-chunks with the stationary w1 tile reused
    across both 512-wide moving slabs; FFN2 uses h^T tiles as the
    stationary and w2 stripes as the moving operand, producing y
    ROW-major directly into PSUM (no output transpose at all).
"""

import numpy as np

import concourse.bass as bass
import concourse.tile as tile
from concourse import bacc
from concourse import mybir

F32 = mybir.dt.float32
BF16 = mybir.dt.bfloat16
EXP = mybir.ActivationFunctionType.Exp
RELU = mybir.ActivationFunctionType.Relu
SQRT = mybir.ActivationFunctionType.Sqrt
SQUARE = mybir.ActivationFunctionType.Square
IDENT = mybir.ActivationFunctionType.Identity
ADD = mybir.AluOpType.add
SUB = mybir.AluOpType.subtract
MUL = mybir.AluOpType.mult

LN_EPS = 1e-5
E = 64          # head dim
W = 80          # head dim + ones column + pad to xbar 16-row granularity
P = 128         # partitions


def build_program(S=2048, D=1024, F=4096):
    """Per-core program.  S = full seq len; queries are rows [0, Lq)."""
    H = D // E
    Lq = S // 2
    ST = S // P          # key tiles
    LT = Lq // P         # query row tiles
    DT = D // P          # d chunks
    FT = F // P          # f chunks
    NSL = 2
    SL = Lq // NSL       # moving slab width (512)
    GS = min(512, D)     # bn_stats subgroup size

    nc = bacc.Bacc("TRN2")

    xb = nc.dram_tensor("xb", [S, D], F32, kind="ExternalInput")
    xb16 = nc.dram_tensor("xb16", [S, D], BF16, kind="ExternalInput")
    w1s = nc.dram_tensor("w1s", [FT, P, D], BF16, kind="ExternalInput")
    w2s = nc.dram_tensor("w2s", [FT, P, D], BF16, kind="ExternalInput")
    b1 = nc.dram_tensor("b1", [F], F32, kind="ExternalInput")
    b2 = nc.dram_tensor("b2", [D], F32, kind="ExternalInput")
    g1h = nc.dram_tensor("g1h", [D], BF16, kind="ExternalInput")
    be1h = nc.dram_tensor("be1h", [D], BF16, kind="ExternalInput")
    g2 = nc.dram_tensor("g2", [D], F32, kind="ExternalInput")
    be2 = nc.dram_tensor("be2", [D], F32, kind="ExternalInput")
    out = nc.dram_tensor("out", [Lq, D], F32, kind="ExternalOutput")

    def bcast(dram_vec, n):
        a = dram_vec[:]
        return bass.AP(tensor=a.tensor, offset=a.offset, ap=[[0, P]] + a.ap)

    with tile.TileContext(nc) as tc:
        with (
            tc.tile_pool(name="persist", bufs=1) as persist,
            tc.tile_pool(name="small", bufs=6) as small,
            tc.tile_pool(name="gb", bufs=1) as gbp,
            tc.tile_pool(name="resp", bufs=2) as resp,
        ):
            # PE warmup spin: dense junk matmuls while the input DMAs land.
            # Keeps the tensor engine 100% busy from t=0 so the DVFS boost
            # (2.4 GHz) engages before the real attention work starts.
            with (
                tc.tile_pool(name="warm", bufs=1) as wp,
                tc.tile_pool(name="warmp", bufs=1, space="PSUM") as wpp,
            ):
                wdum = wp.tile([P, SL], BF16)
                nc.gpsimd.memset(wdum, 0.0)
                wps = wpp.tile([P, SL], F32)
                for _ in range(56):
                    nc.tensor.matmul(wps, wdum[:, 0:P], wdum,
                                     start=True, stop=True)

            b1s = persist.tile([P, FT], F32)
            nc.sync.dma_start(out=b1s, in_=b1[:].rearrange("(t p) -> p t", p=P))
            epst = persist.tile([P, 1], F32)
            nc.vector.memset(epst, LN_EPS)
            # x1 (post-LN1) in bf16: residual-2 source and FFN1 input
            x1b = persist.tile([P, LT, D], BF16)
            # x1^T: [p, lt, dc, j] = x1[lt*128+j, dc*128+p]
            x1T = persist.tile([P, LT, DT, P], BF16)

            # ---------------- attention ----------------
            with (
                tc.tile_pool(name="attn_sb", bufs=1) as asb,
                tc.tile_pool(name="xrp", bufs=2) as xrp,
                tc.tile_pool(name="etp", bufs=4) as etp,
                tc.tile_pool(name="utsp", bufs=2) as utsp,
                tc.tile_pool(name="usp", bufs=3) as usp,
                tc.tile_pool(name="recp", bufs=4) as recp,
                tc.tile_pool(name="lnsc", bufs=3) as lnsc,
            ):
                # attention output, bf16 (residual add upcasts later)
                new_x = asb.tile([P, LT, D], BF16)
                # x^T tiles: [P, DT, S]; d-chunk t holds heads 2t, 2t+1.
                # Interleave the transposes with the vaug row loads so the
                # first heads' operands land as early as possible.
                xT = asb.tile([P, DT, S], BF16)
                vaug = asb.tile([P, ST, H, W], BF16)
                # query rows (bf16) kept for the LN1 residual
                xrows = asb.tile([P, LT, D], BF16)
                nc.gpsimd.memset(vaug[:, :, :, E:W], 0.0)
                nc.gpsimd.memset(vaug[:, :, :, E:E + 1], 1.0)
                nc.sync.dma_start_transpose(
                    out=xT[:, 0, :], in_=xb16[:, 0:P])
                for u in range(ST):
                    if u < LT:
                        xr = xrows[:, u, :]
                    else:
                        xr = xrp.tile([P, D], BF16, tag="xr")
                    nc.sync.dma_start(out=xr, in_=xb16[u * P:(u + 1) * P, :])
                    nc.vector.tensor_copy(
                        out=vaug[:, u, :, 0:E],
                        in_=xr.rearrange("p (h e) -> p h e", e=E))
                    if u % 2 == 1 and 1 + u // 2 < DT:
                        t = 1 + u // 2
                        nc.sync.dma_start_transpose(
                            out=xT[:, t, :], in_=xb16[:, t * P:(t + 1) * P])

                g1b = gbp.tile([P, D], BF16, tag="g1")
                nc.gpsimd.dma_start(out=g1b, in_=bcast(g1h, D))
                be1b = gbp.tile([P, D], BF16, tag="be1")
                nc.gpsimd.dma_start(out=be1b, in_=bcast(be1h, D))

                def head_epilogue(h, ut):
                    uts = utsp.tile([W, Lq], BF16, name="uts", tag="uts")
                    nc.vector.tensor_copy(out=uts, in_=ut)
                    # U: [p, lt, w] = U^T[w, lt*128+p]
                    us = usp.tile([P, LT, W], BF16, name="us", tag="us")
                    nc.sync.dma_start_transpose(out=us, in_=uts)
                    rec = recp.tile([P, LT], F32, name="rec", tag="rec")
                    nc.vector.reciprocal(out=rec, in_=us[:, :, E])
                    for lt in range(LT):
                        nc.vector.tensor_scalar_mul(
                            out=new_x[:, lt, h * E:(h + 1) * E],
                            in0=us[:, lt, 0:E],
                            scalar1=rec[:, lt:lt + 1])

                with (
                    tc.tile_pool(name="scp", bufs=2, space="PSUM") as scp,
                    tc.tile_pool(name="utp", bufs=2, space="PSUM") as utp,
                ):
                    for h in range(H):
                        t, ro = h // 2, (h % 2) * E
                        ut = utp.tile([W, Lq], F32, name="ut", tag="ut")

                        def emit_ut(et_u, ut=ut, h=h):
                            et_p, u_p = et_u
                            for s in range(NSL):
                                nc.tensor.matmul(
                                    ut[:, s * SL:(s + 1) * SL],
                                    vaug[:, u_p, h, :],
                                    et_p[:, s * SL:(s + 1) * SL],
                                    start=(u_p == 0), stop=(u_p == ST - 1))

                        # software pipeline: AV lags TWO chunks so every PE
                        # instruction's exp input is long done -> the PE
                        # issues back-to-back with no semaphore stalls
                        pend = []
                        for u in range(ST):
                            sc = scp.tile([P, Lq], F32, name="sc", tag="sc")
                            et = etp.tile([P, Lq], BF16, name="et", tag="et")
                            for s in range(NSL):
                                nc.tensor.matmul(
                                    sc[:, s * SL:(s + 1) * SL],
                                    xT[ro:ro + E, t, u * P:(u + 1) * P],
                                    xT[ro:ro + E, t, s * SL:(s + 1) * SL],
                                    start=True, stop=True)
                            if len(pend) >= 2:
                                emit_ut(pend.pop(0))
                            nc.scalar.activation(
                                out=et, in_=sc, func=EXP, scale=1.0 / 8.0)
                            pend.append((et, u))
                        for p_ in pend:
                            emit_ut(p_)
                        head_epilogue(h, ut)

                # residual 1 + LN1 -> x1b (bf16) and x1T (dma transpose).
                # Work is spread over DVE / ACT / GpSimd and pipelined
                # across row tiles to shorten the attention->FFN boundary.
                for lt in range(LT):
                    xs = resp.tile([P, D], F32, tag="res")
                    rs = small.tile([P, 1], F32, tag="rs")
                    nc.vector.scalar_tensor_tensor(
                        out=xs, in0=new_x[:, lt, :], scalar=1.0,
                        in1=xrows[:, lt, :], op0=MUL, op1=ADD, accum_out=rs)
                    mean = small.tile([P, 1], F32, tag="mean")
                    nc.vector.tensor_scalar_mul(
                        out=mean, in0=rs, scalar1=1.0 / D)
                    sq = lnsc.tile([P, D], F32, tag="sq", bufs=1)
                    ssq = small.tile([P, 1], F32, tag="ssq")
                    nc.scalar.activation(
                        out=sq, in_=xs, func=SQUARE, accum_out=ssq)
                    nmsq = small.tile([P, 1], F32, tag="nmsq")
                    # var = ssq/D - mean^2  (+eps under the sqrt)
                    nc.vector.scalar_tensor_tensor(
                        out=nmsq, in0=mean, scalar=-1.0,
                        in1=mean, op0=MUL, op1=MUL)
                    var = small.tile([P, 1], F32, tag="var")
                    nc.vector.tensor_scalar(
                        out=var, in0=ssq, scalar1=1.0 / D, scalar2=nmsq,
                        op0=MUL, op1=ADD)
                    rstd = small.tile([P, 1], F32, tag="rstd")
                    nc.scalar.activation(
                        out=rstd, in_=var, func=SQRT, bias=epst)
                    nc.vector.reciprocal(out=rstd, in_=rstd)
                    nmr = small.tile([P, 1], F32, tag="nmr")
                    nc.vector.scalar_tensor_tensor(
                        out=nmr, in0=mean, scalar=-1.0,
                        in1=rstd, op0=MUL, op1=MUL)
                    xh = lnsc.tile([P, D], BF16, tag="xh")
                    nc.scalar.activation(
                        out=xh, in_=xs, func=IDENT, scale=rstd, bias=nmr)
                    # all-bf16 SBUF operands -> DVE 4x mode, ~0.3us each
                    nc.vector.tensor_mul(out=xh, in0=xh, in1=g1b)
                    nc.vector.tensor_add(out=x1b[:, lt, :], in0=xh, in1=be1b)
                    nc.sync.dma_start_transpose(
                        out=x1T[:, lt, :, :], in_=x1b[:, lt, :])

            # ---------------- FFN ----------------
            with (
                tc.tile_pool(name="ffn_sb", bufs=1) as fsb,
                tc.tile_pool(name="w1p", bufs=3) as w1p,
                tc.tile_pool(name="otp", bufs=2) as otp,
            ):
                g2b = gbp.tile([P, D], F32, tag="g2")
                nc.gpsimd.dma_start(out=g2b, in_=bcast(g2, D))
                be2b = gbp.tile([P, D], F32, tag="be2")
                nc.gpsimd.dma_start(out=be2b, in_=bcast(be2, D))
                b2b = gbp.tile([P, D], F32, tag="b2")
                nc.gpsimd.dma_start(out=b2b, in_=bcast(b2, D))

                # all w2 stripes + all h^T tiles stay resident
                w2a = fsb.tile([P, FT, D], BF16)
                for j in range(FT):
                    nc.sync.dma_start(out=w2a[:, j, :], in_=w2s[j])
                hts = fsb.tile([P, FT, Lq], BF16)

                # FFN1: h^T[f, l] = relu(w1 x1^T + b1)
                with tc.tile_pool(name="hpp", bufs=4, space="PSUM") as hpp:
                    for ft in range(FT):
                        wt = w1p.tile([P, D], BF16, tag="w1")
                        nc.sync.dma_start(out=wt, in_=w1s[ft])
                        hp = [hpp.tile([P, SL], F32, name=f"hp{s}",
                                       tag=f"hp{s}")
                              for s in range(NSL)]
                        # first two f-chunks: finish slab 0 (query rows
                        # 0..511) before touching slab 1, so FFN1 starts as
                        # soon as LN1 of the first 4 row tiles lands
                        if ft < 2:
                            loop = [(s, dc) for s in range(NSL)
                                    for dc in range(DT)]
                        else:
                            loop = [(s, dc) for dc in range(DT)
                                    for s in range(NSL)]
                        for s, dc in loop:
                            nc.tensor.matmul(
                                hp[s],
                                wt[:, dc * P:(dc + 1) * P],
                                x1T[:, s * (LT // NSL):(s + 1) * (LT // NSL), dc, :],
                                start=(dc == 0), stop=(dc == DT - 1))
                        for s in range(NSL):
                            nc.scalar.activation(
                                out=hts[:, ft, s * SL:(s + 1) * SL],
                                in_=hp[s], func=RELU,
                                bias=b1s[:, ft:ft + 1])

                # FFN2: y[l, d] = sum_j (h^T_j)^T w2_j  (row-major output)
                with tc.tile_pool(name="ypp", bufs=2, space="PSUM") as ypp:
                    for lt in range(LT):
                        yp = ypp.tile([P, D], F32)
                        for j in range(FT):
                            for s in range(NSL):
                                nc.tensor.matmul(
                                    yp[:, s * SL:(s + 1) * SL],
                                    hts[:, j, lt * P:(lt + 1) * P],
                                    w2a[:, j, s * SL:(s + 1) * SL],
                                    start=(j == 0), stop=(j == FT - 1))
                        # residual 2 + b2 + LN2 -> out
                        xs = resp.tile([P, D], F32, tag="res")
                        nc.vector.scalar_tensor_tensor(
                            out=xs, in0=yp, scalar=1.0,
                            in1=x1b[:, lt, :], op0=MUL, op1=ADD)
                        nc.vector.tensor_add(out=xs, in0=xs, in1=b2b)
                        ot = otp.tile([P, D], F32, tag="ot")
                        _layer_norm(nc, small, ot, xs, g2b, be2b, epst, GS)
                        nc.sync.dma_start(
                            out=out[lt * P:(lt + 1) * P, :], in_=ot)

    nc.finalize()
    return nc


def _layer_norm(nc, small, out_ap, x_ap, gb, beb, epst, GS):
    """out = (x - mean(x)) * rsqrt(var(x) + eps) * g + be over free dim.
    x_ap is clobbered (normalized in place); out_ap gets the final value
    and may have a different dtype."""
    D = x_ap.shape[-1]
    ngr = D // GS
    st = small.tile([P, ngr, 6], F32, tag="bnst")
    xg = x_ap.rearrange("p (g k) -> p g k", k=GS)
    for g in range(ngr):
        nc.vector.bn_stats(out=st[:, g, :], in_=xg[:, g, :])
    mv = small.tile([P, 2], F32, tag="bnmv")
    nc.vector.bn_aggr(out=mv, in_=st)
    rstd = small.tile([P, 1], F32, tag="rstd")
    nc.scalar.activation(out=rstd, in_=mv[:, 1:2], func=SQRT, bias=epst)
    nc.vector.reciprocal(out=rstd, in_=rstd)
    nc.vector.tensor_scalar(
        out=x_ap, in0=x_ap, scalar1=mv[:, 0:1], scalar2=rstd,
        op0=SUB, op1=MUL)
    nc.vector.tensor_mul(out=x_ap, in0=x_ap, in1=gb)
    nc.vector.tensor_add(out=out_ap, in0=x_ap, in1=beb)


# ---------------------------------------------------------------------------
# host side
# ---------------------------------------------------------------------------

_PROG_CACHE = {}


def get_program(S=2048, D=1024, F=4096):
    key = (S, D, F)
    if key not in _PROG_CACHE:
        _PROG_CACHE[key] = build_program(S, D, F)
    return _PROG_CACHE[key]


def make_in_maps(x, w1, b1, w2, b2, g1, be1, g2, be2, n_cores=8):
    B, L, D = x.shape
    F = w1.shape[0]
    Lq = L // 2
    DT, FT = D // 128, F // 128
    import ml_dtypes
    # w1s[ft, p, dc*128+f] = w1[ft*128+f, dc*128+p]
    w1s = np.ascontiguousarray(
        w1.reshape(FT, 128, DT, 128).transpose(0, 3, 2, 1)
        .reshape(FT, 128, D)).astype(ml_dtypes.bfloat16)
    # w2s[j, p, d] = w2[d, j*128+p]
    w2s = np.ascontiguousarray(
        w2.T.reshape(FT, 128, D)).astype(ml_dtypes.bfloat16)
    common = dict(w1s=w1s, w2s=w2s, b1=b1, b2=b2,
                  g1h=np.asarray(g1, ml_dtypes.bfloat16),
                  be1h=np.asarray(be1, ml_dtypes.bfloat16),
                  g2=g2, be2=be2)
    in_maps = []
    for c in range(n_cores):
        b, half = c // 2, c % 2
        lo = half * Lq
        xq = x[b, lo:lo + Lq]
        xo = x[b, Lq - lo:2 * Lq - lo]
        xbl = np.ascontiguousarray(np.concatenate([xq, xo], axis=0))
        in_maps.append(dict(xb=xbl, xb16=xbl.astype(ml_dtypes.bfloat16),
                            **common))
    return in_maps


def kernel(x, w1, b1, w2, b2, g1, be1, g2, be2):
    from concourse.bass_utils import run_bass_kernel_spmd

    x = np.asarray(x, dtype=np.float32)
    B, L, D = x.shape
    F = w1.shape[0]
    Lq = L // 2
    n_cores = 2 * B
    nc = get_program(L, D, F)
    in_maps = make_in_maps(x, np.asarray(w1, np.float32), np.asarray(b1, np.float32),
                           np.asarray(w2, np.float32), np.asarray(b2, np.float32),
                           np.asarray(g1, np.float32), np.asarray(be1, np.float32),
                           np.asarray(g2, np.float32), np.asarray(be2, np.float32),
                           n_cores)
    res = run_bass_kernel_spmd(nc, in_maps, core_ids=list(range(n_cores)))
    outp = np.empty((B, L, D), dtype=np.float32)
    for c in range(n_cores):
        b, half = c // 2, c % 2
        outp[b, half * Lq:(half + 1) * Lq] = res.results[c]["out"]
    return outp


# revision 37
# speedup vs baseline: 1.2617x; 1.0066x over previous
"""Trainium2 Bass kernel for a dense transformer encoder layer.

Reference computation (per batch b):
    q = x.reshape(L, H, E)                       # H=16 heads, E=64
    scores = q @ q^T per head, scaled softmax    # A = softmax(s/8)
    new_x  = concat_h(A_h @ q_h)                 # [L, D]
    x1 = LN(x + new_x; g1, be1)
    y  = relu(x1 @ w1^T + b1) @ w2^T + b2
    out = LN(x1 + y; g2, be2)

Sharding: pure data parallel over (batch, seq-half): core c handles
batch c//2, query rows [(c%2)*1024, +1024).  Keys/values span the full
sequence of that batch, so every core gets the whole x[b] (queries
reordered first) and the full FFN weights.  No device collectives.

v2 design notes (all matmuls bf16, PE does ONLY matmuls):
  - x^T, U^T, x1^T are produced with DMA xbar transposes (16-bit dtype,
    src partition %16, free %128), not PE transposes.
  - scores are computed TRANSPOSED ([s, l]) so exp(scores^T) is the
    moving operand of the AV matmul; V carries a ones column so the
    softmax denominator rides along in row 64 of U^T (rows 65..79 pad
    to the xbar 16-row granularity with zero columns).
  - FFN weights are streamed from HBM once, as one [128, 1024] stripe
    per 128-row block (64 DMAs total instead of 1024 tile DMAs).
  - FFN1 accumulates over d-chunks with the stationary w1 tile reused
    across both 512-wide moving slabs; FFN2 uses h^T tiles as the
    stationary and w2 stripes as the moving operand, producing y
    ROW-major directly into PSUM (no output transpose at all).
"""

import numpy as np

import concourse.bass as bass
import concourse.tile as tile
from concourse import bacc
from concourse import mybir

F32 = mybir.dt.float32
BF16 = mybir.dt.bfloat16
EXP = mybir.ActivationFunctionType.Exp
RELU = mybir.ActivationFunctionType.Relu
SQRT = mybir.ActivationFunctionType.Sqrt
SQUARE = mybir.ActivationFunctionType.Square
IDENT = mybir.ActivationFunctionType.Identity
ADD = mybir.AluOpType.add
SUB = mybir.AluOpType.subtract
MUL = mybir.AluOpType.mult

LN_EPS = 1e-5
E = 64          # head dim
W = 80          # head dim + ones column + pad to xbar 16-row granularity
P = 128         # partitions


def build_program(S=2048, D=1024, F=4096):
    """Per-core program.  S = full seq len; queries are rows [0, Lq)."""
    H = D // E
    Lq = S // 2
    ST = S // P          # key tiles
    LT = Lq // P         # query row tiles
    DT = D // P          # d chunks
    FT = F // P          # f chunks
    NSL = 2
    SL = Lq // NSL       # moving slab width (512)
    GS = min(512, D)     # bn_stats subgroup size

    nc = bacc.Bacc("TRN2")

    xb = nc.dram_tensor("xb", [S, D], F32, kind="ExternalInput")
    xb16 = nc.dram_tensor("xb16", [S, D], BF16, kind="ExternalInput")
    w1s = nc.dram_tensor("w1s", [FT, P, D], BF16, kind="ExternalInput")
    w2s = nc.dram_tensor("w2s", [FT, P, D], BF16, kind="ExternalInput")
    b1 = nc.dram_tensor("b1", [F], F32, kind="ExternalInput")
    b2 = nc.dram_tensor("b2", [D], F32, kind="ExternalInput")
    g1h = nc.dram_tensor("g1h", [D], BF16, kind="ExternalInput")
    be1h = nc.dram_tensor("be1h", [D], BF16, kind="ExternalInput")
    g2 = nc.dram_tensor("g2", [D], F32, kind="ExternalInput")
    be2 = nc.dram_tensor("be2", [D], F32, kind="ExternalInput")
    out = nc.dram_tensor("out", [Lq, D], F32, kind="ExternalOutput")

    def bcast(dram_vec, n):
        a = dram_vec[:]
        return bass.AP(tensor=a.tensor, offset=a.offset, ap=[[0, P]] + a.ap)

    with tile.TileContext(nc) as tc:
        with (
            tc.tile_pool(name="persist", bufs=1) as persist,
            tc.tile_pool(name="small", bufs=6) as small,
            tc.tile_pool(name="gb", bufs=1) as gbp,
            tc.tile_pool(name="resp", bufs=2) as resp,
        ):
            # PE warmup spin: dense junk matmuls while the input DMAs land.
            # Keeps the tensor engine 100% busy from t=0 so the DVFS boost
            # (2.4 GHz) engages before the real attention work starts.
            with (
                tc.tile_pool(name="warm", bufs=1) as wp,
                tc.tile_pool(name="warmp", bufs=1, space="PSUM") as wpp,
            ):
                wdum = wp.tile([P, SL], BF16)
                nc.gpsimd.memset(wdum, 0.0)
                wps = wpp.tile([P, SL], F32)
                for _ in range(32):
                    nc.tensor.matmul(wps, wdum[:, 0:P], wdum,
                                     start=True, stop=True)

            b1s = persist.tile([P, FT], F32)
            nc.sync.dma_start(out=b1s, in_=b1[:].rearrange("(t p) -> p t", p=P))
            epst = persist.tile([P, 1], F32)
            nc.vector.memset(epst, LN_EPS)
            # x1 (post-LN1) in bf16: residual-2 source and FFN1 input
            x1b = persist.tile([P, LT, D], BF16)
            # x1^T: [p, lt, dc, j] = x1[lt*128+j, dc*128+p]
            x1T = persist.tile([P, LT, DT, P], BF16)

            # ---------------- attention ----------------
            with (
                tc.tile_pool(name="attn_sb", bufs=1) as asb,
                tc.tile_pool(name="xrp", bufs=2) as xrp,
                tc.tile_pool(name="etp", bufs=4) as etp,
                tc.tile_pool(name="utsp", bufs=2) as utsp,
                tc.tile_pool(name="usp", bufs=3) as usp,
                tc.tile_pool(name="recp", bufs=4) as recp,
                tc.tile_pool(name="lnsc", bufs=3) as lnsc,
            ):
                # attention output, bf16 (residual add upcasts later)
                new_x = asb.tile([P, LT, D], BF16)
                # x^T tiles: [P, DT, S]; d-chunk t holds heads 2t, 2t+1.
                # Interleave the transposes with the vaug row loads so the
                # first heads' operands land as early as possible.
                xT = asb.tile([P, DT, S], BF16)
                vaug = asb.tile([P, ST, H, W], BF16)
                # query rows (bf16) kept for the LN1 residual
                xrows = asb.tile([P, LT, D], BF16)
                nc.gpsimd.memset(vaug[:, :, :, E:W], 0.0)
                nc.gpsimd.memset(vaug[:, :, :, E:E + 1], 1.0)
                nc.sync.dma_start_transpose(
                    out=xT[:, 0, :], in_=xb16[:, 0:P])
                for u in range(ST):
                    if u < LT:
                        xr = xrows[:, u, :]
                    else:
                        xr = xrp.tile([P, D], BF16, tag="xr")
                    nc.sync.dma_start(out=xr, in_=xb16[u * P:(u + 1) * P, :])
                    nc.vector.tensor_copy(
                        out=vaug[:, u, :, 0:E],
                        in_=xr.rearrange("p (h e) -> p h e", e=E))
                    if u % 2 == 1 and 1 + u // 2 < DT:
                        t = 1 + u // 2
                        nc.sync.dma_start_transpose(
                            out=xT[:, t, :], in_=xb16[:, t * P:(t + 1) * P])

                g1b = gbp.tile([P, D], BF16, tag="g1")
                nc.gpsimd.dma_start(out=g1b, in_=bcast(g1h, D))
                be1b = gbp.tile([P, D], BF16, tag="be1")
                nc.gpsimd.dma_start(out=be1b, in_=bcast(be1h, D))

                def head_epilogue(h, ut):
                    uts = utsp.tile([W, Lq], BF16, name="uts", tag="uts")
                    nc.vector.tensor_copy(out=uts, in_=ut)
                    # U: [p, lt, w] = U^T[w, lt*128+p]
                    us = usp.tile([P, LT, W], BF16, name="us", tag="us")
                    nc.sync.dma_start_transpose(out=us, in_=uts)
                    rec = recp.tile([P, LT], F32, name="rec", tag="rec")
                    nc.vector.reciprocal(out=rec, in_=us[:, :, E])
                    for lt in range(LT):
                        nc.vector.tensor_scalar_mul(
                            out=new_x[:, lt, h * E:(h + 1) * E],
                            in0=us[:, lt, 0:E],
                            scalar1=rec[:, lt:lt + 1])

                with (
                    tc.tile_pool(name="scp", bufs=2, space="PSUM") as scp,
                    tc.tile_pool(name="utp", bufs=2, space="PSUM") as utp,
                ):
                    for h in range(H):
                        t, ro = h // 2, (h % 2) * E
                        ut = utp.tile([W, Lq], F32, name="ut", tag="ut")

                        def emit_ut(et_u, ut=ut, h=h):
                            et_p, u_p = et_u
                            for s in range(NSL):
                                nc.tensor.matmul(
                                    ut[:, s * SL:(s + 1) * SL],
                                    vaug[:, u_p, h, :],
                                    et_p[:, s * SL:(s + 1) * SL],
                                    start=(u_p == 0), stop=(u_p == ST - 1))

                        # software pipeline: AV lags TWO chunks so every PE
                        # instruction's exp input is long done -> the PE
                        # issues back-to-back with no semaphore stalls
                        pend = []
                        for u in range(ST):
                            sc = scp.tile([P, Lq], F32, name="sc", tag="sc")
                            et = etp.tile([P, Lq], BF16, name="et", tag="et")
                            for s in range(NSL):
                                nc.tensor.matmul(
                                    sc[:, s * SL:(s + 1) * SL],
                                    xT[ro:ro + E, t, u * P:(u + 1) * P],
                                    xT[ro:ro + E, t, s * SL:(s + 1) * SL],
                                    start=True, stop=True)
                            if len(pend) >= 2:
                                emit_ut(pend.pop(0))
                            nc.scalar.activation(
                                out=et, in_=sc, func=EXP, scale=1.0 / 8.0)
                            pend.append((et, u))
                        for p_ in pend:
                            emit_ut(p_)
                        head_epilogue(h, ut)

                # residual 1 + LN1 -> x1b (bf16) and x1T (dma transpose).
                # Work is spread over DVE / ACT / GpSimd and pipelined
                # across row tiles to shorten the attention->FFN boundary.
                for lt in range(LT):
                    xs = resp.tile([P, D], F32, tag="res")
                    rs = small.tile([P, 1], F32, tag="rs")
                    nc.vector.scalar_tensor_tensor(
                        out=xs, in0=new_x[:, lt, :], scalar=1.0,
                        in1=xrows[:, lt, :], op0=MUL, op1=ADD, accum_out=rs)
                    mean = small.tile([P, 1], F32, tag="mean")
                    nc.vector.tensor_scalar_mul(
                        out=mean, in0=rs, scalar1=1.0 / D)
                    sq = lnsc.tile([P, D], F32, tag="sq", bufs=1)
                    ssq = small.tile([P, 1], F32, tag="ssq")
                    nc.scalar.activation(
                        out=sq, in_=xs, func=SQUARE, accum_out=ssq)
                    nmsq = small.tile([P, 1], F32, tag="nmsq")
                    # var = ssq/D - mean^2  (+eps under the sqrt)
                    nc.vector.scalar_tensor_tensor(
                        out=nmsq, in0=mean, scalar=-1.0,
                        in1=mean, op0=MUL, op1=MUL)
                    var = small.tile([P, 1], F32, tag="var")
                    nc.vector.tensor_scalar(
                        out=var, in0=ssq, scalar1=1.0 / D, scalar2=nmsq,
                        op0=MUL, op1=ADD)
                    rstd = small.tile([P, 1], F32, tag="rstd")
                    nc.scalar.activation(
                        out=rstd, in_=var, func=SQRT, bias=epst)
                    nc.vector.reciprocal(out=rstd, in_=rstd)
                    nmr = small.tile([P, 1], F32, tag="nmr")
                    nc.vector.scalar_tensor_tensor(
                        out=nmr, in0=mean, scalar=-1.0,
                        in1=rstd, op0=MUL, op1=MUL)
                    xh = lnsc.tile([P, D], BF16, tag="xh")
                    nc.scalar.activation(
                        out=xh, in_=xs, func=IDENT, scale=rstd, bias=nmr)
                    # all-bf16 SBUF operands -> DVE 4x mode, ~0.3us each
                    nc.vector.tensor_mul(out=xh, in0=xh, in1=g1b)
                    nc.vector.tensor_add(out=x1b[:, lt, :], in0=xh, in1=be1b)
                    nc.sync.dma_start_transpose(
                        out=x1T[:, lt, :, :], in_=x1b[:, lt, :])

            # ---------------- FFN ----------------
            with (
                tc.tile_pool(name="ffn_sb", bufs=1) as fsb,
                tc.tile_pool(name="w1p", bufs=2) as w1p,
                tc.tile_pool(name="otp", bufs=2) as otp,
            ):
                g2b = gbp.tile([P, D], F32, tag="g2")
                nc.gpsimd.dma_start(out=g2b, in_=bcast(g2, D))
                be2b = gbp.tile([P, D], F32, tag="be2")
                nc.gpsimd.dma_start(out=be2b, in_=bcast(be2, D))
                b2b = gbp.tile([P, D], F32, tag="b2")
                nc.gpsimd.dma_start(out=b2b, in_=bcast(b2, D))

                # all w2 stripes + all h^T tiles stay resident
                w2a = fsb.tile([P, FT, D], BF16)
                for j in range(FT):
                    nc.sync.dma_start(out=w2a[:, j, :], in_=w2s[j])
                hts = fsb.tile([P, FT, Lq], BF16)

                # FFN1: h^T[f, l] = relu(w1 x1^T + b1)
                with tc.tile_pool(name="hpp", bufs=4, space="PSUM") as hpp:
                    for ft in range(FT):
                        wt = w1p.tile([P, D], BF16, tag="w1")
                        nc.sync.dma_start(out=wt, in_=w1s[ft])
                        hp = [hpp.tile([P, SL], F32, name=f"hp{s}",
                                       tag=f"hp{s}")
                              for s in range(NSL)]
                        # first two f-chunks: finish slab 0 (query rows
                        # 0..511) before touching slab 1, so FFN1 starts as
                        # soon as LN1 of the first 4 row tiles lands
                        if ft < 2:
                            loop = [(s, dc) for s in range(NSL)
                                    for dc in range(DT)]
                        else:
                            loop = [(s, dc) for dc in range(DT)
                                    for s in range(NSL)]
                        for s, dc in loop:
                            nc.tensor.matmul(
                                hp[s],
                                wt[:, dc * P:(dc + 1) * P],
                                x1T[:, s * (LT // NSL):(s + 1) * (LT // NSL), dc, :],
                                start=(dc == 0), stop=(dc == DT - 1))
                        for s in range(NSL):
                            nc.scalar.activation(
                                out=hts[:, ft, s * SL:(s + 1) * SL],
                                in_=hp[s], func=RELU,
                                bias=b1s[:, ft:ft + 1])

                # FFN2: y[l, d] = sum_j (h^T_j)^T w2_j  (row-major output)
                with tc.tile_pool(name="ypp", bufs=3, space="PSUM") as ypp:
                    for lt in range(LT):
                        yp = ypp.tile([P, D], F32)
                        for j in range(FT):
                            for s in range(NSL):
                                nc.tensor.matmul(
                                    yp[:, s * SL:(s + 1) * SL],
                                    hts[:, j, lt * P:(lt + 1) * P],
                                    w2a[:, j, s * SL:(s + 1) * SL],
                                    start=(j == 0), stop=(j == FT - 1))
                        # residual 2 + b2, with the row sum riding along
                        xs = resp.tile([P, D], F32, tag="res")
                        nc.vector.scalar_tensor_tensor(
                            out=xs, in0=yp, scalar=1.0,
                            in1=x1b[:, lt, :], op0=MUL, op1=ADD)
                        rs = small.tile([P, 1], F32, tag="rs2")
                        nc.vector.scalar_tensor_tensor(
                            out=xs, in0=xs, scalar=1.0,
                            in1=b2b, op0=MUL, op1=ADD, accum_out=rs)
                        # LN2 via ACT square-accum variance (DVE stays light)
                        mean = small.tile([P, 1], F32, tag="mean2")
                        nc.vector.tensor_scalar_mul(
                            out=mean, in0=rs, scalar1=1.0 / D)
                        sq = resp.tile([P, D], F32, tag="sq2", bufs=2)
                        ssq = small.tile([P, 1], F32, tag="ssq2")
                        nc.scalar.activation(
                            out=sq, in_=xs, func=SQUARE, accum_out=ssq)
                        nmsq = small.tile([P, 1], F32, tag="nmsq2")
                        nc.vector.scalar_tensor_tensor(
                            out=nmsq, in0=mean, scalar=-1.0,
                            in1=mean, op0=MUL, op1=MUL)
                        var = small.tile([P, 1], F32, tag="var2")
                        nc.vector.tensor_scalar(
                            out=var, in0=ssq, scalar1=1.0 / D, scalar2=nmsq,
                            op0=MUL, op1=ADD)
                        rstd = small.tile([P, 1], F32, tag="rstd2")
                        nc.scalar.activation(
                            out=rstd, in_=var, func=SQRT, bias=epst)
                        nc.vector.reciprocal(out=rstd, in_=rstd)
                        nmr = small.tile([P, 1], F32, tag="nmr2")
                        nc.vector.scalar_tensor_tensor(
                            out=nmr, in0=mean, scalar=-1.0,
                            in1=rstd, op0=MUL, op1=MUL)
                        xh2 = resp.tile([P, D], F32, tag="sq2", bufs=2,
                                        name="xh2")
                        nc.scalar.activation(
                            out=xh2, in_=xs, func=IDENT, scale=rstd, bias=nmr)
                        nc.vector.tensor_mul(out=xh2, in0=xh2, in1=g2b)
                        ot = otp.tile([P, D], F32, tag="ot")
                        nc.vector.tensor_add(out=ot, in0=xh2, in1=be2b)
                        nc.sync.dma_start(
                            out=out[lt * P:(lt + 1) * P, :], in_=ot)

    nc.finalize()
    return nc


def _layer_norm(nc, small, out_ap, x_ap, gb, beb, epst, GS):
    """out = (x - mean(x)) * rsqrt(var(x) + eps) * g + be over free dim.
    x_ap is clobbered (normalized in place); out_ap gets the final value
    and may have a different dtype."""
    D = x_ap.shape[-1]
    ngr = D // GS
    st = small.tile([P, ngr, 6], F32, tag="bnst")
    xg = x_ap.rearrange("p (g k) -> p g k", k=GS)
    for g in range(ngr):
        nc.vector.bn_stats(out=st[:, g, :], in_=xg[:, g, :])
    mv = small.tile([P, 2], F32, tag="bnmv")
    nc.vector.bn_aggr(out=mv, in_=st)
    rstd = small.tile([P, 1], F32, tag="rstd")
    nc.scalar.activation(out=rstd, in_=mv[:, 1:2], func=SQRT, bias=epst)
    nc.vector.reciprocal(out=rstd, in_=rstd)
    nc.vector.tensor_scalar(
        out=x_ap, in0=x_ap, scalar1=mv[:, 0:1], scalar2=rstd,
        op0=SUB, op1=MUL)
    nc.vector.tensor_mul(out=x_ap, in0=x_ap, in1=gb)
    nc.vector.tensor_add(out=out_ap, in0=x_ap, in1=beb)


# ---------------------------------------------------------------------------
# host side
# ---------------------------------------------------------------------------

_PROG_CACHE = {}


def get_program(S=2048, D=1024, F=4096):
    key = (S, D, F)
    if key not in _PROG_CACHE:
        _PROG_CACHE[key] = build_program(S, D, F)
    return _PROG_CACHE[key]


def make_in_maps(x, w1, b1, w2, b2, g1, be1, g2, be2, n_cores=8):
    B, L, D = x.shape
    F = w1.shape[0]
    Lq = L // 2
    DT, FT = D // 128, F // 128
    import ml_dtypes
    # w1s[ft, p, dc*128+f] = w1[ft*128+f, dc*128+p]
    w1s = np.ascontiguousarray(
        w1.reshape(FT, 128, DT, 128).transpose(0, 3, 2, 1)
        .reshape(FT, 128, D)).astype(ml_dtypes.bfloat16)
    # w2s[j, p, d] = w2[d, j*128+p]
    w2s = np.ascontiguousarray(
        w2.T.reshape(FT, 128, D)).astype(ml_dtypes.bfloat16)
    common = dict(w1s=w1s, w2s=w2s, b1=b1, b2=b2,
                  g1h=np.asarray(g1, ml_dtypes.bfloat16),
                  be1h=np.asarray(be1, ml_dtypes.bfloat16),
                  g2=g2, be2=be2)
    in_maps = []
    for c in range(n_cores):
        b, half = c // 2, c % 2
        lo = half * Lq
        xq = x[b, lo:lo + Lq]
        xo = x[b, Lq - lo:2 * Lq - lo]
        xbl = np.ascontiguousarray(np.concatenate([xq, xo], axis=0))
        in_maps.append(dict(xb=xbl, xb16=xbl.astype(ml_dtypes.bfloat16),
                            **common))
    return in_maps


def kernel(x, w1, b1, w2, b2, g1, be1, g2, be2):
    from concourse.bass_utils import run_bass_kernel_spmd

    x = np.asarray(x, dtype=np.float32)
    B, L, D = x.shape
    F = w1.shape[0]
    Lq = L // 2
    n_cores = 2 * B
    nc = get_program(L, D, F)
    in_maps = make_in_maps(x, np.asarray(w1, np.float32), np.asarray(b1, np.float32),
                           np.asarray(w2, np.float32), np.asarray(b2, np.float32),
                           np.asarray(g1, np.float32), np.asarray(be1, np.float32),
                           np.asarray(g2, np.float32), np.asarray(be2, np.float32),
                           n_cores)
    res = run_bass_kernel_spmd(nc, in_maps, core_ids=list(range(n_cores)))
    outp = np.empty((B, L, D), dtype=np.float32)
    for c in range(n_cores):
        b, half = c // 2, c % 2
        outp[b, half * Lq:(half + 1) * Lq] = res.results[c]["out"]
    return outp


# revision 38
# speedup vs baseline: 1.3131x; 1.0407x over previous
"""Trainium2 Bass kernel for a dense transformer encoder layer.

Reference computation (per batch b):
    q = x.reshape(L, H, E)                       # H=16 heads, E=64
    scores = q @ q^T per head, scaled softmax    # A = softmax(s/8)
    new_x  = concat_h(A_h @ q_h)                 # [L, D]
    x1 = LN(x + new_x; g1, be1)
    y  = relu(x1 @ w1^T + b1) @ w2^T + b2
    out = LN(x1 + y; g2, be2)

Sharding: pure data parallel over (batch, seq-half): core c handles
batch c//2, query rows [(c%2)*1024, +1024).  Keys/values span the full
sequence of that batch, so every core gets the whole x[b] (queries
reordered first) and the full FFN weights.  No device collectives.

v2 design notes (all matmuls bf16, PE does ONLY matmuls):
  - x^T, U^T, x1^T are produced with DMA xbar transposes (16-bit dtype,
    src partition %16, free %128), not PE transposes.
  - scores are computed TRANSPOSED ([s, l]) so exp(scores^T) is the
    moving operand of the AV matmul; V carries a ones column so the
    softmax denominator rides along in row 64 of U^T (rows 65..79 pad
    to the xbar 16-row granularity with zero columns).
  - FFN weights are streamed from HBM once, as one [128, 1024] stripe
    per 128-row block (64 DMAs total instead of 1024 tile DMAs).
  - FFN1 accumulates over d-chunks with the stationary w1 tile reused
    across both 512-wide moving slabs; FFN2 uses h^T tiles as the
    stationary and w2 stripes as the moving operand, producing y
    ROW-major directly into PSUM (no output transpose at all).
"""

import numpy as np

import concourse.bass as bass
import concourse.tile as tile
from concourse import bacc
from concourse import mybir

F32 = mybir.dt.float32
BF16 = mybir.dt.bfloat16
EXP = mybir.ActivationFunctionType.Exp
RELU = mybir.ActivationFunctionType.Relu
SQRT = mybir.ActivationFunctionType.Sqrt
SQUARE = mybir.ActivationFunctionType.Square
IDENT = mybir.ActivationFunctionType.Identity
ADD = mybir.AluOpType.add
SUB = mybir.AluOpType.subtract
MUL = mybir.AluOpType.mult

LN_EPS = 1e-5
E = 64          # head dim
W = 80          # head dim + ones column + pad to xbar 16-row granularity
P = 128         # partitions


def build_program(S=2048, D=1024, F=4096):
    """Per-core program.  S = full seq len; queries are rows [0, Lq)."""
    H = D // E
    Lq = S // 2
    ST = S // P          # key tiles
    LT = Lq // P         # query row tiles
    DT = D // P          # d chunks
    FT = F // P          # f chunks
    NSL = 2
    SL = Lq // NSL       # moving slab width (512)
    GS = min(512, D)     # bn_stats subgroup size

    nc = bacc.Bacc("TRN2")

    xb = nc.dram_tensor("xb", [S, D], F32, kind="ExternalInput")
    xb16 = nc.dram_tensor("xb16", [S, D], BF16, kind="ExternalInput")
    w1s = nc.dram_tensor("w1s", [FT, P, D], BF16, kind="ExternalInput")
    w2s = nc.dram_tensor("w2s", [FT, P, D], BF16, kind="ExternalInput")
    b1 = nc.dram_tensor("b1", [F], F32, kind="ExternalInput")
    b2 = nc.dram_tensor("b2", [D], F32, kind="ExternalInput")
    g1h = nc.dram_tensor("g1h", [D], BF16, kind="ExternalInput")
    be1h = nc.dram_tensor("be1h", [D], BF16, kind="ExternalInput")
    g2 = nc.dram_tensor("g2", [D], F32, kind="ExternalInput")
    be2 = nc.dram_tensor("be2", [D], F32, kind="ExternalInput")
    out = nc.dram_tensor("out", [Lq, D], F32, kind="ExternalOutput")

    def bcast(dram_vec, n):
        a = dram_vec[:]
        return bass.AP(tensor=a.tensor, offset=a.offset, ap=[[0, P]] + a.ap)

    with tile.TileContext(nc) as tc:
        with (
            tc.tile_pool(name="persist", bufs=1) as persist,
            tc.tile_pool(name="small", bufs=6) as small,
            tc.tile_pool(name="gb", bufs=1) as gbp,
            tc.tile_pool(name="resp", bufs=2) as resp,
        ):
            # PE warmup spin: dense junk matmuls while the input DMAs land.
            # Keeps the tensor engine 100% busy from t=0 so the DVFS boost
            # (2.4 GHz) engages before the real attention work starts.
            with (
                tc.tile_pool(name="warm", bufs=1) as wp,
                tc.tile_pool(name="warmp", bufs=1, space="PSUM") as wpp,
            ):
                wdum = wp.tile([P, SL], BF16)
                nc.gpsimd.memset(wdum, 0.0)
                wps = wpp.tile([P, SL], F32)
                for _ in range(32):
                    nc.tensor.matmul(wps, wdum[:, 0:P], wdum,
                                     start=True, stop=True)

            b1s = persist.tile([P, FT], F32)
            nc.sync.dma_start(out=b1s, in_=b1[:].rearrange("(t p) -> p t", p=P))
            epst = persist.tile([P, 1], F32)
            nc.vector.memset(epst, LN_EPS)
            # x1 (post-LN1) in bf16: residual-2 source and FFN1 input
            x1b = persist.tile([P, LT, D], BF16)
            # x1^T: [p, lt, dc, j] = x1[lt*128+j, dc*128+p]
            x1T = persist.tile([P, LT, DT, P], BF16)

            # ---------------- attention ----------------
            with (
                tc.tile_pool(name="attn_sb", bufs=1) as asb,
                tc.tile_pool(name="xrp", bufs=2) as xrp,
                tc.tile_pool(name="etp", bufs=4) as etp,
                tc.tile_pool(name="utsp", bufs=2) as utsp,
                tc.tile_pool(name="usp", bufs=3) as usp,
                tc.tile_pool(name="recp", bufs=4) as recp,
                tc.tile_pool(name="lnsc", bufs=3) as lnsc,
            ):
                # attention output, bf16 (residual add upcasts later)
                new_x = asb.tile([P, LT, D], BF16)
                # x^T tiles: [P, DT, S]; d-chunk t holds heads 2t, 2t+1.
                # Interleave the transposes with the vaug row loads so the
                # first heads' operands land as early as possible.
                xT = asb.tile([P, DT, S], BF16)
                vaug = asb.tile([P, ST, H, W], BF16)
                # query rows (bf16) kept for the LN1 residual
                xrows = asb.tile([P, LT, D], BF16)
                nc.gpsimd.memset(vaug[:, :, :, E:W], 0.0)
                nc.gpsimd.memset(vaug[:, :, :, E:E + 1], 1.0)
                nc.sync.dma_start_transpose(
                    out=xT[:, 0, :], in_=xb16[:, 0:P])
                for u in range(ST):
                    if u < LT:
                        xr = xrows[:, u, :]
                    else:
                        xr = xrp.tile([P, D], BF16, tag="xr")
                    nc.sync.dma_start(out=xr, in_=xb16[u * P:(u + 1) * P, :])
                    nc.vector.tensor_copy(
                        out=vaug[:, u, :, 0:E],
                        in_=xr.rearrange("p (h e) -> p h e", e=E))
                    if u % 2 == 1 and 1 + u // 2 < DT:
                        t = 1 + u // 2
                        nc.sync.dma_start_transpose(
                            out=xT[:, t, :], in_=xb16[:, t * P:(t + 1) * P])

                g1b = gbp.tile([P, D], BF16, tag="g1")
                nc.gpsimd.dma_start(out=g1b, in_=bcast(g1h, D))
                be1b = gbp.tile([P, D], BF16, tag="be1")
                nc.gpsimd.dma_start(out=be1b, in_=bcast(be1h, D))

                def head_epilogue(h, ut):
                    uts = utsp.tile([W, Lq], BF16, name="uts", tag="uts")
                    nc.vector.tensor_copy(out=uts, in_=ut)
                    # U: [p, lt, w] = U^T[w, lt*128+p]
                    us = usp.tile([P, LT, W], BF16, name="us", tag="us")
                    nc.sync.dma_start_transpose(out=us, in_=uts)
                    rec = recp.tile([P, LT], F32, name="rec", tag="rec")
                    nc.vector.reciprocal(out=rec, in_=us[:, :, E])
                    for lt in range(LT):
                        nc.vector.tensor_scalar_mul(
                            out=new_x[:, lt, h * E:(h + 1) * E],
                            in0=us[:, lt, 0:E],
                            scalar1=rec[:, lt:lt + 1])

                with (
                    tc.tile_pool(name="scp", bufs=2, space="PSUM") as scp,
                    tc.tile_pool(name="utp", bufs=2, space="PSUM") as utp,
                ):
                    for h in range(H):
                        t, ro = h // 2, (h % 2) * E
                        ut = utp.tile([W, Lq], F32, name="ut", tag="ut")

                        def emit_ut(et_u, ut=ut, h=h):
                            et_p, u_p = et_u
                            for s in range(NSL):
                                nc.tensor.matmul(
                                    ut[:, s * SL:(s + 1) * SL],
                                    vaug[:, u_p, h, :],
                                    et_p[:, s * SL:(s + 1) * SL],
                                    start=(u_p == 0), stop=(u_p == ST - 1))

                        # software pipeline: AV lags TWO chunks so every PE
                        # instruction's exp input is long done -> the PE
                        # issues back-to-back with no semaphore stalls
                        pend = []
                        for u in range(ST):
                            sc = scp.tile([P, Lq], F32, name="sc", tag="sc")
                            et = etp.tile([P, Lq], BF16, name="et", tag="et")
                            for s in range(NSL):
                                nc.tensor.matmul(
                                    sc[:, s * SL:(s + 1) * SL],
                                    xT[ro:ro + E, t, u * P:(u + 1) * P],
                                    xT[ro:ro + E, t, s * SL:(s + 1) * SL],
                                    start=True, stop=True)
                            if len(pend) >= 2:
                                emit_ut(pend.pop(0))
                            nc.scalar.activation(
                                out=et, in_=sc, func=EXP, scale=1.0 / 8.0)
                            pend.append((et, u))
                        for p_ in pend:
                            emit_ut(p_)
                        head_epilogue(h, ut)

                # residual 1 + LN1 -> x1b (bf16) and x1T (dma transpose).
                # Work is spread over DVE / ACT / GpSimd and pipelined
                # across row tiles to shorten the attention->FFN boundary.
                for lt in range(LT):
                    xs = resp.tile([P, D], F32, tag="res")
                    rs = small.tile([P, 1], F32, tag="rs")
                    nc.vector.scalar_tensor_tensor(
                        out=xs, in0=new_x[:, lt, :], scalar=1.0,
                        in1=xrows[:, lt, :], op0=MUL, op1=ADD, accum_out=rs)
                    mean = small.tile([P, 1], F32, tag="mean")
                    nc.vector.tensor_scalar_mul(
                        out=mean, in0=rs, scalar1=1.0 / D)
                    sq = lnsc.tile([P, D], F32, tag="sq", bufs=1)
                    ssq = small.tile([P, 1], F32, tag="ssq")
                    nc.scalar.activation(
                        out=sq, in_=xs, func=SQUARE, accum_out=ssq)
                    nmsq = small.tile([P, 1], F32, tag="nmsq")
                    # var = ssq/D - mean^2  (+eps under the sqrt)
                    nc.vector.scalar_tensor_tensor(
                        out=nmsq, in0=mean, scalar=-1.0,
                        in1=mean, op0=MUL, op1=MUL)
                    var = small.tile([P, 1], F32, tag="var")
                    nc.vector.tensor_scalar(
                        out=var, in0=ssq, scalar1=1.0 / D, scalar2=nmsq,
                        op0=MUL, op1=ADD)
                    rstd = small.tile([P, 1], F32, tag="rstd")
                    nc.scalar.activation(
                        out=rstd, in_=var, func=SQRT, bias=epst)
                    nc.vector.reciprocal(out=rstd, in_=rstd)
                    nmr = small.tile([P, 1], F32, tag="nmr")
                    nc.vector.scalar_tensor_tensor(
                        out=nmr, in0=mean, scalar=-1.0,
                        in1=rstd, op0=MUL, op1=MUL)
                    xh = lnsc.tile([P, D], BF16, tag="xh")
                    nc.scalar.activation(
                        out=xh, in_=xs, func=IDENT, scale=rstd, bias=nmr)
                    # all-bf16 SBUF operands -> DVE 4x mode, ~0.3us each
                    nc.vector.tensor_mul(out=xh, in0=xh, in1=g1b)
                    nc.vector.tensor_add(out=x1b[:, lt, :], in0=xh, in1=be1b)
                    nc.sync.dma_start_transpose(
                        out=x1T[:, lt, :, :], in_=x1b[:, lt, :])

            # ---------------- FFN ----------------
            with (
                tc.tile_pool(name="ffn_sb", bufs=1) as fsb,
                tc.tile_pool(name="w1p", bufs=2) as w1p,
                tc.tile_pool(name="otp", bufs=2) as otp,
            ):
                g2b = gbp.tile([P, D], F32, tag="g2")
                nc.gpsimd.dma_start(out=g2b, in_=bcast(g2, D))
                be2b = gbp.tile([P, D], F32, tag="be2")
                nc.gpsimd.dma_start(out=be2b, in_=bcast(be2, D))
                b2b = gbp.tile([P, D], F32, tag="b2")
                nc.gpsimd.dma_start(out=b2b, in_=bcast(b2, D))

                # all w2 stripes + all h^T tiles stay resident; w2 loads go
                # on the (idle) gpsimd DMA queue so they never delay the w1
                # stripes that gate FFN1's first matmuls
                w2a = fsb.tile([P, FT, D], BF16)
                for j in range(FT):
                    nc.gpsimd.dma_start(out=w2a[:, j, :], in_=w2s[j])
                hts = fsb.tile([P, FT, Lq], BF16)

                # FFN1: h^T[f, l] = relu(w1 x1^T + b1)
                with tc.tile_pool(name="hpp", bufs=4, space="PSUM") as hpp:
                    for ft in range(FT):
                        wt = w1p.tile([P, D], BF16, tag="w1")
                        nc.sync.dma_start(out=wt, in_=w1s[ft])
                        hp = [hpp.tile([P, SL], F32, name=f"hp{s}",
                                       tag=f"hp{s}")
                              for s in range(NSL)]
                        # first two f-chunks: finish slab 0 (query rows
                        # 0..511) before touching slab 1, so FFN1 starts as
                        # soon as LN1 of the first 4 row tiles lands
                        if ft < 2:
                            loop = [(s, dc) for s in range(NSL)
                                    for dc in range(DT)]
                        else:
                            loop = [(s, dc) for dc in range(DT)
                                    for s in range(NSL)]
                        for s, dc in loop:
                            nc.tensor.matmul(
                                hp[s],
                                wt[:, dc * P:(dc + 1) * P],
                                x1T[:, s * (LT // NSL):(s + 1) * (LT // NSL), dc, :],
                                start=(dc == 0), stop=(dc == DT - 1))
                        for s in range(NSL):
                            nc.scalar.activation(
                                out=hts[:, ft, s * SL:(s + 1) * SL],
                                in_=hp[s], func=RELU,
                                bias=b1s[:, ft:ft + 1])

                # FFN2: y[l, d] = sum_j (h^T_j)^T w2_j  (row-major output)
                with tc.tile_pool(name="ypp", bufs=3, space="PSUM") as ypp:
                    for lt in range(LT):
                        yp = ypp.tile([P, D], F32)
                        for j in range(FT):
                            for s in range(NSL):
                                nc.tensor.matmul(
                                    yp[:, s * SL:(s + 1) * SL],
                                    hts[:, j, lt * P:(lt + 1) * P],
                                    w2a[:, j, s * SL:(s + 1) * SL],
                                    start=(j == 0), stop=(j == FT - 1))
                        # residual 2 + b2, with the row sum riding along
                        xs = resp.tile([P, D], F32, tag="res")
                        nc.vector.scalar_tensor_tensor(
                            out=xs, in0=yp, scalar=1.0,
                            in1=x1b[:, lt, :], op0=MUL, op1=ADD)
                        rs = small.tile([P, 1], F32, tag="rs2")
                        nc.vector.scalar_tensor_tensor(
                            out=xs, in0=xs, scalar=1.0,
                            in1=b2b, op0=MUL, op1=ADD, accum_out=rs)
                        # LN2 via ACT square-accum variance (DVE stays light)
                        mean = small.tile([P, 1], F32, tag="mean2")
                        nc.vector.tensor_scalar_mul(
                            out=mean, in0=rs, scalar1=1.0 / D)
                        sq = resp.tile([P, D], F32, tag="sq2", bufs=2)
                        ssq = small.tile([P, 1], F32, tag="ssq2")
                        nc.scalar.activation(
                            out=sq, in_=xs, func=SQUARE, accum_out=ssq)
                        nmsq = small.tile([P, 1], F32, tag="nmsq2")
                        nc.vector.scalar_tensor_tensor(
                            out=nmsq, in0=mean, scalar=-1.0,
                            in1=mean, op0=MUL, op1=MUL)
                        var = small.tile([P, 1], F32, tag="var2")
                        nc.vector.tensor_scalar(
                            out=var, in0=ssq, scalar1=1.0 / D, scalar2=nmsq,
                            op0=MUL, op1=ADD)
                        rstd = small.tile([P, 1], F32, tag="rstd2")
                        nc.scalar.activation(
                            out=rstd, in_=var, func=SQRT, bias=epst)
                        nc.vector.reciprocal(out=rstd, in_=rstd)
                        nmr = small.tile([P, 1], F32, tag="nmr2")
                        nc.vector.scalar_tensor_tensor(
                            out=nmr, in0=mean, scalar=-1.0,
                            in1=rstd, op0=MUL, op1=MUL)
                        xh2 = resp.tile([P, D], F32, tag="sq2", bufs=2,
                                        name="xh2")
                        nc.scalar.activation(
                            out=xh2, in_=xs, func=IDENT, scale=rstd, bias=nmr)
                        nc.vector.tensor_mul(out=xh2, in0=xh2, in1=g2b)
                        ot = otp.tile([P, D], F32, tag="ot")
                        nc.vector.tensor_add(out=ot, in0=xh2, in1=be2b)
                        nc.sync.dma_start(
                            out=out[lt * P:(lt + 1) * P, :], in_=ot)

    nc.finalize()
    return nc


def _layer_norm(nc, small, out_ap, x_ap, gb, beb, epst, GS):
    """out = (x - mean(x)) * rsqrt(var(x) + eps) * g + be over free dim.
    x_ap is clobbered (normalized in place); out_ap gets the final value
    and may have a different dtype."""
    D = x_ap.shape[-1]
    ngr = D // GS
    st = small.tile([P, ngr, 6], F32, tag="bnst")
    xg = x_ap.rearrange("p (g k) -> p g k", k=GS)
    for g in range(ngr):
        nc.vector.bn_stats(out=st[:, g, :], in_=xg[:, g, :])
    mv = small.tile([P, 2], F32, tag="bnmv")
    nc.vector.bn_aggr(out=mv, in_=st)
    rstd = small.tile([P, 1], F32, tag="rstd")
    nc.scalar.activation(out=rstd, in_=mv[:, 1:2], func=SQRT, bias=epst)
    nc.vector.reciprocal(out=rstd, in_=rstd)
    nc.vector.tensor_scalar(
        out=x_ap, in0=x_ap, scalar1=mv[:, 0:1], scalar2=rstd,
        op0=SUB, op1=MUL)
    nc.vector.tensor_mul(out=x_ap, in0=x_ap, in1=gb)
    nc.vector.tensor_add(out=out_ap, in0=x_ap, in1=beb)


# ---------------------------------------------------------------------------
# host side
# ---------------------------------------------------------------------------

_PROG_CACHE = {}


def get_program(S=2048, D=1024, F=4096):
    key = (S, D, F)
    if key not in _PROG_CACHE:
        _PROG_CACHE[key] = build_program(S, D, F)
    return _PROG_CACHE[key]


def make_in_maps(x, w1, b1, w2, b2, g1, be1, g2, be2, n_cores=8):
    B, L, D = x.shape
    F = w1.shape[0]
    Lq = L // 2
    DT, FT = D // 128, F // 128
    import ml_dtypes
    # w1s[ft, p, dc*128+f] = w1[ft*128+f, dc*128+p]
    w1s = np.ascontiguousarray(
        w1.reshape(FT, 128, DT, 128).transpose(0, 3, 2, 1)
        .reshape(FT, 128, D)).astype(ml_dtypes.bfloat16)
    # w2s[j, p, d] = w2[d, j*128+p]
    w2s = np.ascontiguousarray(
        w2.T.reshape(FT, 128, D)).astype(ml_dtypes.bfloat16)
    common = dict(w1s=w1s, w2s=w2s, b1=b1, b2=b2,
                  g1h=np.asarray(g1, ml_dtypes.bfloat16),
                  be1h=np.asarray(be1, ml_dtypes.bfloat16),
                  g2=g2, be2=be2)
    in_maps = []
    for c in range(n_cores):
        b, half = c // 2, c % 2
        lo = half * Lq
        xq = x[b, lo:lo + Lq]
        xo = x[b, Lq - lo:2 * Lq - lo]
        xbl = np.ascontiguousarray(np.concatenate([xq, xo], axis=0))
        in_maps.append(dict(xb=xbl, xb16=xbl.astype(ml_dtypes.bfloat16),
                            **common))
    return in_maps


def kernel(x, w1, b1, w2, b2, g1, be1, g2, be2):
    from concourse.bass_utils import run_bass_kernel_spmd

    x = np.asarray(x, dtype=np.float32)
    B, L, D = x.shape
    F = w1.shape[0]
    Lq = L // 2
    n_cores = 2 * B
    nc = get_program(L, D, F)
    in_maps = make_in_maps(x, np.asarray(w1, np.float32), np.asarray(b1, np.float32),
                           np.asarray(w2, np.float32), np.asarray(b2, np.float32),
                           np.asarray(g1, np.float32), np.asarray(be1, np.float32),
                           np.asarray(g2, np.float32), np.asarray(be2, np.float32),
                           n_cores)
    res = run_bass_kernel_spmd(nc, in_maps, core_ids=list(range(n_cores)))
    outp = np.empty((B, L, D), dtype=np.float32)
    for c in range(n_cores):
        b, half = c // 2, c % 2
        outp[b, half * Lq:(half + 1) * Lq] = res.results[c]["out"]
    return outp
